# revision 1
# baseline (speedup 1.0000x reference)
"""Trainium (trn2) Bass kernel for a 2-layer GAT over N=100k nodes / E=1.7M edges.

Strategy (node-transform / edge-aggregate split, streamed fp8 selection)
------------------------------------------------------------------------
Edges are sorted by destination on the host (index-only preprocessing); the
destination axis is sharded across the 8 NeuronCores in contiguous 128-node
windows (98 windows per core).  Each GAT layer runs as TWO SPMD kernels with
host-side index gathers (pure permutations / casts - no host FLOPs) between
them:

* node kernel (P0/P2): h = x @ W and the folded attention logits
  al_s = x @ (W a_s), al_d = x @ (W a_d), computed ONCE PER NODE (dense
  matmuls, ~50 us/core).  P2 fuses the ELU of the layer-1 output into its
  input stream and merges the al columns into the main matmul (64+2<=128).
* host: gathers per-edge streams h[src], al_s[src], al_d[dst] into the
  dst-sorted slot order (numpy fancy indexing = permutation only), and
  prebuilds a graph-constant fp8 one-hot selection stream
  S[e, t*128+n] = (rel_dst==n) shared by both layers.
* edge kernel (E1/E2): per 32-tile group, z = al_s+al_d (DVE 2x),
  leaky_relu on ACT, then ONE ACT Exp op writes exp(z-4) broadcast over
  each head's 16 dims in (c,h)-interleaved channel order plus the compact
  denominator block (cols 128:136); the constant -4 bias keeps fp16 exp
  in range and cancels in the softmax.  One in-place DVE multiply (2x
  mode, all unit-stride) forms the messages, and one mixed fp8xfp16
  matmul per 128-edge tile (fp8 stationary -> fast weight load)
  accumulates [msg | exp] into the window's PSUM slot.  E2 (1 head)
  broadcasts exp(z) straight into the multiply and carries the
  denominator on a host-appended ones column.  Epilogues split across
  ACT (PSUM reads, +eps via Copy bias) and DVE (reciprocal, scale).
  Input streams ride the SP hardware DMA queue; output DMAs ride the
  ACT queue so window outputs never head-of-line-block the streams.

The (c,h) channel interleave is free: W1's columns and W2's rows are
permuted on the host.  Edge kernels run at the HBM stream floor (~317
GB/s/core measured): h[src] fp16 + S fp8 + logits = ~48 KB per 128 edges.

Environment workarounds: this container's walrus build allows only ONE
semaphore wait per instruction (split onto nop carriers post-scheduling), and
the GPSIMD ucode libraries are absent (so no dma_gather/indirect-DMA fast
paths - hence the host-gather design).
"""
import numpy as np

import concourse.bass as bass
import concourse.mybir as mybir
import concourse.tile as tile
from concourse.bass_utils import run_bass_kernel_spmd

P = 128
F16 = mybir.dt.float16
F32 = mybir.dt.float32
F8 = mybir.dt.float8e4
AF = mybir.ActivationFunctionType
OP = mybir.AluOpType
NEG_SLOPE = 0.2
EXP_BIAS = -4.0     # exp(z + EXP_BIAS): constant shift cancels in softmax
GRP = 64            # tiles per stream group
PAD_REL = 255.0     # rel value for pad slots -> is_equal never matches
N_CORES = 8
EPS = 1e-30
CH = 512            # node-kernel chunk (one PSUM bank of fp32)

# ------------------------------------------------------------------ patches

_wsplit_counter = [0]


def _split_excess_waits(nc, max_waits=1):
    """This walrus build rejects >1 sem-wait per instruction ("Too many sync
    wait commands"). Move overflow waits onto same-engine nop carriers."""
    n_split = 0
    for f in nc.m.functions:
        for blk in f.blocks:
            changed = False
            out = []
            for inst in blk.instructions:
                si = inst.sync_info
                if si is not None and len(si.on_wait) > max_waits:
                    waits = list(si.on_wait)
                    keep = waits[len(waits) - max_waits:]
                    overflow = waits[: len(waits) - max_waits]
                    for i in range(0, len(overflow), max_waits):
                        _wsplit_counter[0] += 1
                        nop = mybir.InstNoOp(
                            name=f"I-wsplit-{_wsplit_counter[0]}", ins=[], outs=[])
                        nop.engine = inst.engine
                        nop.sync_info = mybir.SyncInfo(
                            on_wait=overflow[i: i + max_waits], on_update=[])
                        out.append(nop)
                    inst.sync_info = mybir.SyncInfo(
                        on_wait=keep, on_update=list(si.on_update))
                    changed = True
                    n_split += 1
                out.append(inst)
            if changed:
                blk.instructions = out
    return n_split


def _finalize_kernel(nc):
    import bass_rust as _bass_rust
    from concourse.library_config import all_libraries, standard
    from concourse.library_overlay import lower_extended_insts

    inst_type_to_lib_mask = {}
    for lib in all_libraries:
        for inst_type in lib.instructions:
            inst_type_to_lib_mask[inst_type] = inst_type_to_lib_mask.get(
                inst_type, 0) | (1 << lib.index)
    _bass_rust.insert_library_loads(
        nc, inst_type_to_lib_mask, len(all_libraries), standard.index)
    lower_extended_insts(nc)
    _split_excess_waits(nc)


# ------------------------------------------------------------------ host prep

class _Graph:
    """Host-side index preprocessing: sort by dst, shard dst windows across
    cores, pad per-window tile counts to a global schedule so all cores run
    one identical SPMD program."""

    def __init__(self, edge_index, n_nodes, n_cores):
        self.N = n_nodes
        self.C = n_cores
        src = np.asarray(edge_index[0], dtype=np.int64)
        dst = np.asarray(edge_index[1], dtype=np.int64)
        perm = np.argsort(dst, kind="stable")
        self.src_s = src[perm].astype(np.int32)
        self.dst_s = dst[perm].astype(np.int32)

        n_win_total = (n_nodes + P - 1) // P
        self.wpc = (n_win_total + n_cores - 1) // n_cores
        self.n_win = self.wpc * n_cores
        self.shard_nodes = self.wpc * P
        self.n_pad = self.n_win * P

        bounds = np.searchsorted(self.dst_s, np.arange(0, self.n_win + 1) * P)
        counts = np.zeros((n_cores, self.wpc), dtype=np.int64)
        for k in range(n_cores):
            for i in range(self.wpc):
                w = k * self.wpc + i
                if w < n_win_total:
                    counts[k, i] = bounds[w + 1] - bounds[w]
        self.PC = np.maximum(np.ceil(counts / P).astype(np.int64).max(axis=0), 1)
        self.T = int(self.PC.sum())

        self.slot_src = np.zeros((n_cores, self.T * P), dtype=np.int32)
        self.slot_dst = np.zeros((n_cores, self.T * P), dtype=np.int32)
        self.slot_rel = np.full((n_cores, self.T * P), int(PAD_REL), dtype=np.int32)
        for k in range(n_cores):
            t0 = 0
            for i in range(self.wpc):
                w = k * self.wpc + i
                cnt = int(counts[k, i])
                if cnt > 0:
                    e0 = bounds[w]
                    sl = t0 * P
                    self.slot_src[k, sl:sl + cnt] = self.src_s[e0:e0 + cnt]
                    self.slot_dst[k, sl:sl + cnt] = self.dst_s[e0:e0 + cnt]
                    self.slot_rel[k, sl:sl + cnt] = self.dst_s[e0:e0 + cnt] - w * P
                t0 += int(self.PC[i])
        self.src2d = self.slot_src.reshape(n_cores, self.T, P)
        self.dst2d = self.slot_dst.reshape(n_cores, self.T, P)
        self.rel2d = self.slot_rel.reshape(n_cores, self.T, P)

    def stream_feat(self, table, core, ones_col=False):
        """[128, T*C] (or T*(C+1) with a trailing ones column per tile):
        col t*C+c of partition e = table[src[slot t,e], c]."""
        T, C = self.T, table.shape[1]
        W = C + 1 if ones_col else C
        out = np.empty((T, P, W), dtype=np.float16)
        out[:, :, :C] = table[self.src2d[core]]
        if ones_col:
            out[:, :, C] = 1.0
        return np.ascontiguousarray(out.transpose(1, 0, 2)).reshape(P, T * W)

    def stream_zs(self, als, ald, core):
        """[128, T*16] f16: per tile [al_s[src] (8) | al_d[dst] (8)]."""
        T = self.T
        z = np.empty((T, P, 16), dtype=np.float16)
        z[:, :, 0:8] = als[self.src2d[core]]
        z[:, :, 8:16] = ald[self.dst2d[core]]
        return np.ascontiguousarray(z.transpose(1, 0, 2)).reshape(P, T * 16)

    def stream_zs2(self, als, ald, core):
        """[128, T*2] f16: per tile [al_s[src], al_d[dst]]."""
        T = self.T
        z = np.empty((T, P, 2), dtype=np.float16)
        z[:, :, 0] = als[self.src2d[core]]
        z[:, :, 1] = ald[self.dst2d[core]]
        return np.ascontiguousarray(z.transpose(1, 0, 2)).reshape(P, T * 2)

    def stream_sel(self, core):
        """[128, T*128] fp8e4m3 one-hot: col t*128+n of partition e is
        1.0 iff rel[t,e] == n. Graph-only; shared by both layers."""
        if not hasattr(self, "_sel"):
            self._sel = {}
        if core not in self._sel:
            import ml_dtypes
            one = np.float32(1.0).astype(ml_dtypes.float8_e4m3).view(np.uint8)
            T = self.T
            arr = np.zeros((T, P, P), dtype=np.uint8)
            rel = self.rel2d[core]
            t_i, e_i = np.nonzero(rel < P)
            arr[t_i, e_i, rel[t_i, e_i]] = one
            self._sel[core] = np.ascontiguousarray(
                arr.transpose(1, 0, 2)).reshape(P, T * P).view(
                    ml_dtypes.float8_e4m3)
        return self._sel[core]


# ------------------------------------------------------------------ builders

def _build_node(SH, c_in, m_h, m_al, elu, bias_in, bench_loop=1):
    """Per-node transform: hT = (elu?(xT+b)) @ w, alT = same @ wal.
    When m_h+m_al <= 128 the two matmuls merge into one (w carries the
    al columns and alT is folded into hT's extra rows)."""
    merged = (m_h + m_al) <= P
    M = m_h + m_al if merged else m_h
    nc = bass.Bass()
    xT = nc.dram_tensor("xT", [c_in, SH], F16, kind="ExternalInput")
    w = nc.dram_tensor("w", [c_in, M], F16, kind="ExternalInput")
    if not merged:
        wal = nc.dram_tensor("wal", [c_in, m_al], F16, kind="ExternalInput")
    if bias_in:
        bvec = nc.dram_tensor("bvec", [c_in, 1], F32, kind="ExternalInput")
    hT = nc.dram_tensor("hT", [M, SH], F16, kind="ExternalOutput")
    if not merged:
        alT = nc.dram_tensor("alT", [m_al, SH], F16, kind="ExternalOutput")

    with tile.TileContext(nc) as tc:
        with (
            tc.tile_pool(name="const", bufs=1) as constp,
            tc.tile_pool(name="xs", bufs=6) as xsp,
            tc.tile_pool(name="work", bufs=6) as workp,
            tc.tile_pool(name="out", bufs=6) as outp,
            tc.tile_pool(name="psH", bufs=4, space="PSUM") as psH,
            tc.tile_pool(name="psA", bufs=4, space="PSUM") as psA,
        ):
            w_sb = constp.tile([c_in, M], F16)
            nc.sync.dma_start(out=w_sb[:], in_=w[:])
            if not merged:
                wal_sb = constp.tile([c_in, m_al], F16)
                nc.sync.dma_start(out=wal_sb[:], in_=wal[:])
            if bias_in:
                b_sb = constp.tile([c_in, 1], F32)
                nc.sync.dma_start(out=b_sb[:], in_=bvec[:])

            def body(_iv=None):
                for c0 in range(0, SH, CH):
                    nb = min(CH, SH - c0)
                    xc = xsp.tile([c_in, CH], F16, tag="xc")
                    nc.sync.dma_start(out=xc[:, :nb], in_=xT[:, c0:c0 + nb])
                    rhs = xc
                    if elu:
                        if bias_in:
                            nc.vector.tensor_scalar(
                                xc[:, :nb], xc[:, :nb], b_sb[:, 0:1], None,
                                OP.add)
                        mn = workp.tile([c_in, CH], F16, tag="mn")
                        nc.vector.tensor_scalar(
                            mn[:, :nb], xc[:, :nb], 0.0, None, OP.min)
                        nc.scalar.activation(mn[:, :nb], mn[:, :nb], AF.Exp)
                        mx = workp.tile([c_in, CH], F16, tag="mx")
                        nc.vector.tensor_scalar(
                            mx[:, :nb], xc[:, :nb], 0.0, -1.0, OP.max, OP.add)
                        xe = workp.tile([c_in, CH], F16, tag="xe")
                        nc.vector.tensor_tensor(
                            out=xe[:, :nb], in0=mx[:, :nb], in1=mn[:, :nb],
                            op=OP.add)
                        rhs = xe
                    ph = psH.tile([M, CH], F32, tag="ph")
                    nc.tensor.matmul(ph[:, :nb], w_sb[:], rhs[:, :nb],
                                     start=True, stop=True)
                    h_sb = outp.tile([M, CH], F16, tag="h")
                    nc.scalar.activation(h_sb[:, :nb], ph[:, :nb], AF.Copy)
                    nc.scalar.dma_start(out=hT[:, c0:c0 + nb],
                                        in_=h_sb[:, :nb])
                    if not merged:
                        pa = psA.tile([m_al, CH], F32, tag="pa")
                        nc.tensor.matmul(pa[:, :nb], wal_sb[:], rhs[:, :nb],
                                         start=True, stop=True)
                        a_sb = outp.tile([m_al, CH], F16, tag="a")
                        nc.vector.tensor_copy(a_sb[:, :nb], pa[:, :nb])
                        nc.scalar.dma_start(out=alT[:, c0:c0 + nb],
                                            in_=a_sb[:, :nb])

            if bench_loop > 1:
                with tc.For_i(0, bench_loop, 1) as _iv:
                    body(_iv)
            else:
                body()
    _finalize_kernel(nc)
    return nc


def _tile_windows(T, PC, wpc):
    tile_win = []
    for i in range(wpc):
        tile_win += [i] * int(PC[i])
    first_of_win, last_of_win = {}, {}
    for t, w in enumerate(tile_win):
        first_of_win.setdefault(w, t)
        last_of_win[w] = t
    return tile_win, first_of_win, last_of_win


def _build_edge1(T, PC, wpc, bench_loop=1):
    """Layer-1 edge aggregation, 8 heads x 16ch, (c,h)-interleaved channel
    order (channel c*8+h = head h, dim c). Streams h1[src], the fp8 one-hot
    selection matrix, and the logit pairs; one mixed fp8xfp16 matmul per
    128-edge tile accumulates [msg | exp] into the window's PSUM slot.
    Output is the PRE-ELU aggregated feature in (c,h) order."""
    HC, H, ZS, SLOT = 128, 8, 16, 136
    nc = bass.Bass()
    hsrc = nc.dram_tensor("hsrc", [P, T * HC], F16, kind="ExternalInput")
    s8 = nc.dram_tensor("s8", [P, T * P], F8, kind="ExternalInput")
    zs = nc.dram_tensor("zs", [P, T * ZS], F16, kind="ExternalInput")
    out = nc.dram_tensor("out", [wpc * P, HC], F16, kind="ExternalOutput")

    n_groups = (T + GRP - 1) // GRP
    tile_win, first_of_win, last_of_win = _tile_windows(T, PC, wpc)

    with tile.TileContext(nc) as tc:
        with (
            tc.tile_pool(name="const", bufs=1) as constp,
            tc.tile_pool(name="zs", bufs=3) as zsp,
            tc.tile_pool(name="hs", bufs=3) as hsp,
            tc.tile_pool(name="s8", bufs=3) as s8p,
            tc.tile_pool(name="zp", bufs=3) as zpp,
            tc.tile_pool(name="msg", bufs=3) as msgp,
            tc.tile_pool(name="epi", bufs=4) as epip,
            tc.tile_pool(name="psW", bufs=3, space="PSUM") as psW,
        ):
            ebias_sb = constp.tile([P, 1], F32)
            nc.vector.memset(ebias_sb[:], EXP_BIAS)

            def edge_phase(_iv=None):
                psw = None
                for g in range(n_groups):
                    tlo, thi = g * GRP, min(T, g * GRP + GRP)
                    ng = thi - tlo
                    zs_g = zsp.tile([P, GRP * ZS], F16, tag="zs")
                    nc.sync.dma_start(out=zs_g[:, :ng * ZS],
                                      in_=zs[:, tlo * ZS:thi * ZS])
                    hs_g = hsp.tile([P, GRP * HC], F16, tag="hs")
                    nc.sync.dma_start(out=hs_g[:, :ng * HC],
                                      in_=hsrc[:, tlo * HC:thi * HC])
                    s8_g = s8p.tile([P, GRP * P], F8, tag="s8")
                    nc.sync.dma_start(out=s8_g[:, :ng * P],
                                      in_=s8[:, tlo * P:thi * P])

                    zs_r = zs_g[:].rearrange("p (t z) -> p t z", t=GRP)
                    zp_g = zpp.tile([P, GRP * H], F16, tag="zp")
                    zp_r = zp_g[:].rearrange("p (t h) -> p t h", t=GRP)
                    nc.vector.tensor_tensor(
                        out=zp_r[:, :ng, :], in0=zs_r[:, :ng, 0:8],
                        in1=zs_r[:, :ng, 8:16], op=OP.add)
                    nc.scalar.activation(zp_g[:, :ng * H], zp_g[:, :ng * H],
                                         AF.Prelu, alpha=NEG_SLOPE)

                    # ONE ACT op computes exp(z-4) broadcast-expanded over
                    # the 16 dims of each head in (c,h) order, including the
                    # compact denominator block at c=16 (cols 128:136).
                    msg_g = msgp.tile([P, GRP * SLOT], F16, tag="msg")
                    zb = zp_r[:, :ng, :]
                    zp_b = bass.AP(zb.tensor, zb.offset,
                                   [zb.ap[0], zb.ap[1], [0, 17], zb.ap[2]])
                    msg_r = msg_g[:].rearrange("p (t f) -> p t f", t=GRP)
                    mr = msg_r[:, :ng, :]
                    msg_chr = bass.AP(mr.tensor, mr.offset,
                                      [mr.ap[0], mr.ap[1], [8, 17], [1, 8]])
                    nc.scalar.activation(msg_chr, zp_b, AF.Exp,
                                         bias=ebias_sb[:])
                    hs_r = hs_g[:].rearrange("p (t c) -> p t c", t=GRP)
                    nc.vector.tensor_tensor(
                        out=msg_r[:, :ng, 0:HC], in0=hs_r[:, :ng, :],
                        in1=msg_r[:, :ng, 0:HC], op=OP.mult)

                    for j, t in enumerate(range(tlo, thi)):
                        w = tile_win[t]
                        if t == first_of_win[w]:
                            psw = psW.tile([P, SLOT], F32, tag="psw")
                        nc.tensor.matmul(
                            psw[:], s8_g[:, j * P:(j + 1) * P],
                            msg_g[:, j * SLOT:(j + 1) * SLOT],
                            start=(t == first_of_win[w]),
                            stop=(t == last_of_win[w]))
                        if t == last_of_win[w]:
                            den = epip.tile([P, H], F32, tag="den")
                            nc.scalar.activation(den[:], psw[:, HC:HC + H],
                                                 AF.Copy, bias=EPS)
                            rec = epip.tile([P, H], F16, tag="rec")
                            with nc.allow_low_precision(
                                    reason="softmax denominators are O(1)"):
                                nc.vector.reciprocal(rec[:], den[:])
                            o1p = epip.tile([P, HC], F16, tag="o1p")
                            nc.scalar.activation(o1p[:], psw[:, 0:HC],
                                                 AF.Copy)
                            r_ap = rec[:]
                            r_b = bass.AP(r_ap.tensor, r_ap.offset,
                                          [r_ap.ap[0], [0, 16], [1, H]])
                            o1 = epip.tile([P, HC], F16, tag="o1")
                            o1_r = o1[:].rearrange("p (c h) -> p c h", c=16)
                            o1p_r = o1p[:].rearrange("p (c h) -> p c h", c=16)
                            nc.vector.tensor_tensor(
                                out=o1_r, in0=o1p_r, in1=r_b, op=OP.mult)
                            nc.scalar.dma_start(
                                out=out[w * P:(w + 1) * P, :], in_=o1[:])

            if bench_loop > 1:
                with tc.For_i(0, bench_loop, 1) as _iv:
                    edge_phase(_iv)
            else:
                edge_phase()
    _finalize_kernel(nc)
    return nc


def _build_edge2(T, PC, wpc, bias_out, bench_loop=1):
    """Layer-2 edge aggregation, 1 head x 64ch. Messages are the streamed
    h2[src] (with a host-appended ones column for the denominator) scaled
    by the broadcast exp(z); one mixed fp8xfp16 matmul per tile against the
    streamed one-hot selection matrix."""
    C, CW, ZS = 64, 65, 2
    nc = bass.Bass()
    hsrc = nc.dram_tensor("hsrc", [P, T * CW], F16, kind="ExternalInput")
    s8 = nc.dram_tensor("s8", [P, T * P], F8, kind="ExternalInput")
    zs = nc.dram_tensor("zs", [P, T * ZS], F16, kind="ExternalInput")
    if bias_out:
        brep = nc.dram_tensor("brep", [P, C], F32, kind="ExternalInput")
    out = nc.dram_tensor("out", [wpc * P, C], F16, kind="ExternalOutput")

    n_groups = (T + GRP - 1) // GRP
    tile_win, first_of_win, last_of_win = _tile_windows(T, PC, wpc)

    with tile.TileContext(nc) as tc:
        with (
            tc.tile_pool(name="const", bufs=1) as constp,
            tc.tile_pool(name="zs", bufs=3) as zsp,
            tc.tile_pool(name="hs", bufs=3) as hsp,
            tc.tile_pool(name="s8", bufs=3) as s8p,
            tc.tile_pool(name="zp", bufs=3) as zpp,
            tc.tile_pool(name="msg", bufs=3) as msgp,
            tc.tile_pool(name="epi", bufs=4) as epip,
            tc.tile_pool(name="psW", bufs=3, space="PSUM") as psW,
        ):
            ebias_sb = constp.tile([P, 1], F32)
            nc.vector.memset(ebias_sb[:], EXP_BIAS)
            if bias_out:
                brep_sb = constp.tile([P, C], F32)
                nc.sync.dma_start(out=brep_sb[:], in_=brep[:])

            def edge_phase(_iv=None):
                psw = None
                for g in range(n_groups):
                    tlo, thi = g * GRP, min(T, g * GRP + GRP)
                    ng = thi - tlo
                    zs_g = zsp.tile([P, GRP * ZS], F16, tag="zs")
                    nc.sync.dma_start(out=zs_g[:, :ng * ZS],
                                      in_=zs[:, tlo * ZS:thi * ZS])
                    hs_g = hsp.tile([P, GRP * CW], F16, tag="hs")
                    nc.sync.dma_start(out=hs_g[:, :ng * CW],
                                      in_=hsrc[:, tlo * CW:thi * CW])
                    s8_g = s8p.tile([P, GRP * P], F8, tag="s8")
                    nc.sync.dma_start(out=s8_g[:, :ng * P],
                                      in_=s8[:, tlo * P:thi * P])

                    zs_r = zs_g[:].rearrange("p (t z) -> p t z", t=GRP)
                    zp_g = zpp.tile([P, GRP], F16, tag="zp")
                    zp_r = zp_g[:].rearrange("p (t z) -> p t z", z=1)
                    nc.vector.tensor_tensor(
                        out=zp_r[:, :ng], in0=zs_r[:, :ng, 0:1],
                        in1=zs_r[:, :ng, 1:2], op=OP.add)
                    nc.scalar.activation(zp_g[:, :ng], zp_g[:, :ng],
                                         AF.Prelu, alpha=NEG_SLOPE)
                    nc.scalar.activation(zp_g[:, :ng], zp_g[:, :ng], AF.Exp,
                                         bias=ebias_sb[:])

                    # msg = h2src * exp(z) broadcast over the 65 columns
                    msg_g = msgp.tile([P, GRP * CW], F16, tag="msg")
                    msg_r = msg_g[:].rearrange("p (t c) -> p t c", t=GRP)
                    hs_r = hs_g[:].rearrange("p (t c) -> p t c", t=GRP)
                    zb = zp_r[:, :ng]
                    zp_b = bass.AP(zb.tensor, zb.offset,
                                   [zb.ap[0], zb.ap[1], [0, CW]])
                    nc.vector.tensor_tensor(
                        out=msg_r[:, :ng, :], in0=hs_r[:, :ng, :],
                        in1=zp_b, op=OP.mult)

                    for j, t in enumerate(range(tlo, thi)):
                        w = tile_win[t]
                        if t == first_of_win[w]:
                            psw = psW.tile([P, CW], F32, tag="psw")
                        nc.tensor.matmul(
                            psw[:], s8_g[:, j * P:(j + 1) * P],
                            msg_g[:, j * CW:(j + 1) * CW],
                            start=(t == first_of_win[w]),
                            stop=(t == last_of_win[w]))
                        if t == last_of_win[w]:
                            den = epip.tile([P, 1], F32, tag="den")
                            nc.scalar.activation(den[:], psw[:, C:C + 1],
                                                 AF.Copy, bias=EPS)
                            rec = epip.tile([P, 1], F32, tag="rec")
                            nc.vector.reciprocal(rec[:], den[:])
                            r_ap = rec[:]
                            r_b = bass.AP(r_ap.tensor, r_ap.offset,
                                          [r_ap.ap[0], [0, C]])
                            o2 = epip.tile([P, C], F16, tag="o2")
                            nc.vector.tensor_tensor(
                                out=o2[:], in0=psw[:, 0:C], in1=r_b,
                                op=OP.mult)
                            if bias_out:
                                nc.vector.tensor_tensor(
                                    out=o2[:], in0=o2[:], in1=brep_sb[:],
                                    op=OP.add)
                            nc.scalar.dma_start(
                                out=out[w * P:(w + 1) * P, :], in_=o2[:])

            if bench_loop > 1:
                with tc.For_i(0, bench_loop, 1) as _iv:
                    edge_phase(_iv)
            else:
                edge_phase()
    _finalize_kernel(nc)
    return nc


# ------------------------------------------------------------------ runner

def _fold_att(W, a):
    heads, hid = a.shape
    return np.einsum("ihc,hc->ih", W.reshape(W.shape[0], heads, hid), a)


class _GatRunner:
    def __init__(self, n_cores=N_CORES):
        self.C = n_cores
        self._graph = None
        self._graph_key = None
        self._kernels = {}
        self.last_maps = {}

    def graph(self, edge_index, n_nodes):
        key = hash(np.asarray(edge_index).tobytes())
        if key != self._graph_key:
            self._graph = _Graph(edge_index, n_nodes, self.C)
            self._graph_key = key
            self._kernels.clear()
        return self._graph

    def kernel(self, name, bench_loop=1, **kw):
        key = (name, bench_loop, tuple(sorted(kw.items())))
        if key not in self._kernels:
            g = self._graph
            if name.startswith("P"):
                self._kernels[key] = _build_node(
                    g.shard_nodes, bench_loop=bench_loop, **kw)
            elif name == "E1":
                self._kernels[key] = _build_edge1(
                    g.T, g.PC, g.wpc, bench_loop=bench_loop)
            else:
                self._kernels[key] = _build_edge2(
                    g.T, g.PC, g.wpc, bench_loop=bench_loop, **kw)
        return self._kernels[key]

    def _run(self, name, nc, maps):
        self.last_maps[name] = maps
        res = run_bass_kernel_spmd(nc, maps, core_ids=list(range(self.C)))
        return res.results

    def run(self, x, edge_index, W1, a_src1, a_dst1, b1, W2, a_src2, a_dst2,
            b2):
        C = self.C
        N, IN_C = x.shape
        HEADS, HID = a_src1.shape
        HC = HEADS * HID
        OUT_C = W2.shape[1]
        g = self.graph(edge_index, N)
        SH = g.shard_nodes
        # (c,h)-interleaved channel order for the layer-1 hidden features:
        # col c*H+h of h1 holds math channel h*HID+c. Folded into W1's
        # columns (P0) and W2's rows (P2) on the host - pure permutation.
        perm = np.array([(j % HEADS) * HID + j // HEADS
                         for j in range(HC)], dtype=np.int64)

        # ---- P0: per-node h1 / logits --------------------------------
        xT_pad = np.zeros((IN_C, g.n_pad), dtype=np.float16)
        xT_pad[:, :N] = np.asarray(x, np.float32).T
        w1 = np.asarray(W1, np.float32)
        wal1 = np.concatenate(
            [_fold_att(w1, np.asarray(a_src1, np.float32)),
             _fold_att(w1, np.asarray(a_dst1, np.float32))], axis=1)
        mapsP0 = [{"xT": np.ascontiguousarray(xT_pad[:, k * SH:(k + 1) * SH]),
                   "w": np.ascontiguousarray(w1[:, perm]).astype(np.float16),
                   "wal": wal1.astype(np.float16)} for k in range(C)]
        ncP0 = self.kernel("P0", c_in=IN_C, m_h=HC, m_al=2 * HEADS,
                           elu=False, bias_in=False)
        resP0 = self._run("P0", ncP0, mapsP0)
        h1 = np.ascontiguousarray(
            np.concatenate([r["hT"] for r in resP0], axis=1).T)  # [Np,HC] f16
        al1 = np.concatenate([r["alT"] for r in resP0], axis=1)  # [16,Np] f16
        als1 = np.ascontiguousarray(al1[:HEADS].T)
        ald1 = np.ascontiguousarray(al1[HEADS:].T)

        # ---- E1: layer-1 edge aggregation ----------------------------
        mapsE1 = [{"hsrc": g.stream_feat(h1, k),
                   "s8": g.stream_sel(k),
                   "zs": g.stream_zs(als1, ald1, k)} for k in range(C)]
        ncE1 = self.kernel("E1")
        resE1 = self._run("E1", ncE1, mapsE1)
        out1 = np.concatenate([r["out"] for r in resE1], axis=0)  # [Np, HC]

        # ---- P2: ELU + per-node h2 / logits --------------------------
        o1T = np.ascontiguousarray(out1.T)  # [HC, Np] f16, (c,h) rows
        w2 = np.asarray(W2, np.float32)
        wal2 = np.concatenate(
            [_fold_att(w2, np.asarray(a_src2, np.float32)),
             _fold_att(w2, np.asarray(a_dst2, np.float32))], axis=1)
        b1nz = bool(np.any(np.asarray(b1)))
        w2all = np.concatenate([w2[perm], wal2[perm]], axis=1)  # [HC, 66]
        mapsP2 = []
        for k in range(C):
            m = {"xT": np.ascontiguousarray(o1T[:, k * SH:(k + 1) * SH]),
                 "w": w2all.astype(np.float16)}
            if b1nz:
                m["bvec"] = np.asarray(b1, np.float32)[perm].reshape(HC, 1)
            mapsP2.append(m)
        ncP2 = self.kernel("P2", c_in=HC, m_h=OUT_C, m_al=2, elu=True,
                           bias_in=b1nz)
        resP2 = self._run("P2", ncP2, mapsP2)
        h2al = np.concatenate([r["hT"] for r in resP2], axis=1)  # [66, Np]
        h2 = np.ascontiguousarray(h2al[:OUT_C].T)  # [Np, 64] f16
        als2, ald2 = h2al[OUT_C], h2al[OUT_C + 1]

        # ---- E2: layer-2 edge aggregation ----------------------------
        b2nz = bool(np.any(np.asarray(b2)))
        mapsE2 = []
        for k in range(C):
            m = {"hsrc": g.stream_feat(h2, k, ones_col=True),
                 "s8": g.stream_sel(k),
                 "zs": g.stream_zs2(als2, ald2, k)}
            if b2nz:
                m["brep"] = np.tile(np.asarray(b2, np.float32), (P, 1))
            mapsE2.append(m)
        ncE2 = self.kernel("E2", bias_out=b2nz)
        resE2 = self._run("E2", ncE2, mapsE2)
        out2 = np.concatenate([r["out"] for r in resE2], axis=0)
        return out2[:N]


_RUNNER = _GatRunner()


def kernel(x, edge_index, W1, a_src1, a_dst1, b1, W2, a_src2, a_dst2, b2):
    """Full-input / full-output entry point. Returns [N, OUT_C] float32."""
    args = [np.asarray(v) for v in
            (x, edge_index, W1, a_src1, a_dst1, b1, W2, a_src2, a_dst2, b2)]
    return _RUNNER.run(*args).astype(np.float32)



# revision 5
# speedup vs baseline: 1.4739x; 1.4739x over previous
"""Trainium (trn2) Bass kernel for a 2-layer GAT over N=100k nodes / E=1.7M edges.

Strategy (degree-sorted edge grids + identity-stationary PE accumulation)
-------------------------------------------------------------------------
Nodes are sorted by in-degree on the host and packed into windows of 128
similar-degree destination nodes; windows are dealt round-robin across the 8
NeuronCores.  Each window's edges form a dense grid [128 nodes x D slots]
(D = max in-window degree, padded slots carry -inf logits so exp()==0), so
slot j of all 128 nodes is a 128-edge tile whose destination map is the
IDENTITY: the tensor engine accumulates the per-slot message tiles straight
into the window's PSUM bank with a never-changing fp8 identity stationary.
Degree sorting keeps grid padding at ~1.3%, and the one-hot selection stream
of the classic dst-sorted formulation (128 B/edge of pure index overhead)
disappears entirely.

Each GAT layer runs as TWO SPMD kernels with host-side index gathers (pure
permutations / casts - no host FLOPs) between them:

* node kernel (P0/P2): h = x @ W plus folded attention logits computed once
  per node (dense matmuls).  The full per-core input/output panels live in
  SBUF, loaded/stored with a handful of fat DMAs (the previous per-chunk
  1 KB/partition DMAs were latency-bound at ~140 GB/s).
* edge kernel (E1/E2): streams h[src] grids (256/128 B per edge slot) and
  al_src logit grids (16/2 B); al_dst is a tiny per-window constant.  Windows
  are processed in groups (sum of D <= 96) so DVE/ACT run one fat instruction
  per group: DVE adds the logits, ACT applies leaky-relu then writes
  exp(z-4) into the message tile's trailing 8 columns ((c,h)-interleaved
  broadcast for the 8 heads of layer 1, an 8x replica for layer 2's single
  head so the DVE multiply keeps its packed-innermost 2x mode), DVE scales
  the h grid by the exp block, and PE accumulates [msg | exp] per slot.
  Epilogues (PSUM read, reciprocal, scale, output DMA batched per group) are
  emitted one group LATE so the PE/DVE/ACT pipelines never stall on them.
  Streams ride the SP DMA queue; outputs + constants ride the ACT queue.

Measured per-core DMA floor is ~343 GB/s (HBM fair share); the edge kernels
stream ~58.5 MB (E1) / ~28 MB (E2) per core per inference.

Environment workarounds: this container's walrus build allows only ONE
semaphore wait per instruction (split onto nop carriers post-scheduling), and
the GPSIMD ucode libraries are absent (so no dma_gather/indirect-DMA fast
paths - hence the host-gather design).
"""
import numpy as np

import concourse.bass as bass
import concourse.mybir as mybir
import concourse.tile as tile
from concourse.bass_utils import run_bass_kernel_spmd

P = 128
F16 = mybir.dt.float16
F32 = mybir.dt.float32
F8 = mybir.dt.float8e4
AF = mybir.ActivationFunctionType
OP = mybir.AluOpType
NEG_SLOPE = 0.2
EXP_BIAS = -4.0     # exp(z + EXP_BIAS): constant shift cancels in softmax
NEG_INF = -60000.0  # pad-slot logit: exp(lrelu(.)+bias) underflows to 0
N_CORES = 8
EPS = 1e-30
CH = 448            # node-kernel matmul chunk (PSUM: 448*4B <= 2KB bank)
GCAP = 96           # edge-kernel group capacity (sum of window D's)
NWG = 12            # max windows per group

# ------------------------------------------------------------------ patches

_wsplit_counter = [0]


def _split_excess_waits(nc, max_waits=1):
    """This walrus build rejects >1 sem-wait per instruction ("Too many sync
    wait commands"). Move overflow waits onto same-engine nop carriers."""
    n_split = 0
    for f in nc.m.functions:
        for blk in f.blocks:
            changed = False
            out = []
            for inst in blk.instructions:
                si = inst.sync_info
                if si is not None and len(si.on_wait) > max_waits:
                    waits = list(si.on_wait)
                    keep = waits[len(waits) - max_waits:]
                    overflow = waits[: len(waits) - max_waits]
                    for i in range(0, len(overflow), max_waits):
                        _wsplit_counter[0] += 1
                        nop = mybir.InstNoOp(
                            name=f"I-wsplit-{_wsplit_counter[0]}", ins=[], outs=[])
                        nop.engine = inst.engine
                        nop.sync_info = mybir.SyncInfo(
                            on_wait=overflow[i: i + max_waits], on_update=[])
                        out.append(nop)
                    inst.sync_info = mybir.SyncInfo(
                        on_wait=keep, on_update=list(si.on_update))
                    changed = True
                    n_split += 1
                out.append(inst)
            if changed:
                blk.instructions = out
    return n_split


def _finalize_kernel(nc):
    import bass_rust as _bass_rust
    from concourse.library_config import all_libraries, standard
    from concourse.library_overlay import lower_extended_insts

    inst_type_to_lib_mask = {}
    for lib in all_libraries:
        for inst_type in lib.instructions:
            inst_type_to_lib_mask[inst_type] = inst_type_to_lib_mask.get(
                inst_type, 0) | (1 << lib.index)
    _bass_rust.insert_library_loads(
        nc, inst_type_to_lib_mask, len(all_libraries), standard.index)
    lower_extended_insts(nc)
    _split_excess_waits(nc)


# ------------------------------------------------------------------ host prep

class _Graph:
    """Degree-sorted grid preprocessing: sort nodes by in-degree, pack 128
    similar-degree nodes per window, deal windows round-robin across cores
    (slot i of every core shares one padded depth D_i so all cores run one
    identical SPMD program), and scatter each node's edges into its grid row.
    """

    def __init__(self, edge_index, n_nodes, n_cores):
        self.N = n_nodes
        self.C = n_cores
        src = np.asarray(edge_index[0], dtype=np.int64)
        dst = np.asarray(edge_index[1], dtype=np.int64)
        E = src.shape[0]

        deg = np.bincount(dst, minlength=n_nodes)
        order = np.argsort(deg, kind="stable")

        n_win_total = (n_nodes + P - 1) // P
        self.wpc = (n_win_total + n_cores - 1) // n_cores
        n_win = self.wpc * n_cores
        self.n_pad = n_win * P
        self.shard_nodes = self.wpc * P
        n_dummy = self.n_pad - n_nodes

        snode = np.full(self.n_pad, -1, dtype=np.int64)
        snode[n_dummy:] = order                      # ascending degree
        # rows_nodes[k][i, e] = natural node id at (core k, slot i, row e)
        self.rows_nodes = np.ascontiguousarray(
            snode.reshape(self.wpc, n_cores, P).transpose(1, 0, 2))

        wdeg = np.where(snode >= 0, deg[np.clip(snode, 0, None)], 0)
        wmax = wdeg.reshape(self.wpc, n_cores, P).max(axis=2)   # [wpc, cores]
        self.D = np.maximum(wmax.max(axis=1), 1).astype(np.int64)  # [wpc]
        self.off = np.concatenate([[0], np.cumsum(self.D)])
        self.TOT = int(self.D.sum())

        # position of each node in the sorted layout
        posq = np.empty(n_nodes, dtype=np.int64)
        posq[order] = np.arange(n_nodes) + n_dummy

        # scatter edges (dst-sorted, ranked within dst run) into grids
        perm = np.argsort(dst, kind="stable")
        src_s = src[perm]
        dst_s = dst[perm]
        bounds = np.searchsorted(dst_s, np.arange(n_nodes + 1))
        j_e = np.arange(E) - bounds[dst_s]           # rank within dst run
        q_e = posq[dst_s]
        g_e = q_e // P
        row_e = q_e % P
        core_e = g_e % n_cores
        slot_e = g_e // n_cores
        flat_e = self.off[slot_e] + j_e              # grid slot within [TOT]
        self.gidx = np.zeros((n_cores, self.TOT, P), dtype=np.int32)
        self.gidx[core_e, flat_e, row_e] = (src_s + 1).astype(np.int32)

        # group windows: sum(D) <= GCAP, <= NWG windows
        groups = []
        i = 0
        while i < self.wpc:
            i0, sd, nw = i, 0, 0
            while (i < self.wpc and nw < NWG
                   and (nw == 0 or sd + int(self.D[i]) <= GCAP)):
                sd += int(self.D[i])
                i += 1
                nw += 1
            groups.append((i0, nw, int(self.off[i0]), sd))
        self.groups = groups
        self.D_key = tuple(int(d) for d in self.D)

    def stream_h(self, table, core):
        """[128, TOT*C] f16 grid gather: table rows by gidx (0 = zero pad)."""
        C = table.shape[1]
        tp = np.zeros((self.N + 1, C), dtype=np.float16)
        tp[1:] = table
        arr = tp[self.gidx[core]]                    # [TOT, P, C]
        return np.ascontiguousarray(arr.transpose(1, 0, 2)).reshape(
            P, self.TOT * C)

    def stream_als(self, table, core):
        """[128, TOT*H] f16: al_src grid; pad slots -> NEG_INF so exp()==0.
        Dummy rows get one j=0 slot with logit 0 so their softmax denominator
        stays finite (their h rows are zero, so the output row is 0)."""
        H = table.shape[1]
        tp = np.full((self.N + 1, H), NEG_INF, dtype=np.float16)
        tp[1:] = table
        arr = tp[self.gidx[core]]                    # [TOT, P, H]
        i_d, e_d = np.nonzero(self.rows_nodes[core] < 0)
        arr[self.off[i_d], e_d, :] = 0.0
        return np.ascontiguousarray(arr.transpose(1, 0, 2)).reshape(
            P, self.TOT * H)

    def stream_ald(self, table, core):
        """[128, wpc*H] f16: al_dst per (window, row). Dummy rows -> 0."""
        H = table.shape[1]
        tp = np.zeros((self.N + 1, H), dtype=np.float16)
        tp[1:] = table
        arr = tp[self.rows_nodes[core] + 1]          # [wpc, P, H]
        return np.ascontiguousarray(arr.transpose(1, 0, 2)).reshape(
            P, self.wpc * H)

    def ident8(self):
        import ml_dtypes
        return np.eye(P, dtype=np.float32).astype(ml_dtypes.float8_e4m3)


# ------------------------------------------------------------------ builders

def _build_node(SH, c_in, m_h, m_al, elu, bias_in, bench_loop=1):
    """Per-node transform: hT = (elu?(xT+b)) @ w, alT = same @ wal.
    When m_h+m_al <= 128 the two matmuls merge into one.  The whole per-core
    panel is SBUF-resident: quarters stream in with fat DMAs, chunked matmuls
    write a staged output panel, and a few fat DMAs store it."""
    merged = (m_h + m_al) <= P
    M = m_h + m_al if merged else m_h
    QN = 4
    QS = SH // QN
    assert SH % QN == 0 and QS % CH == 0
    nc = bass.Bass()
    xT = nc.dram_tensor("xT", [c_in, SH], F16, kind="ExternalInput")
    w = nc.dram_tensor("w", [c_in, M], F16, kind="ExternalInput")
    if not merged:
        wal = nc.dram_tensor("wal", [c_in, m_al], F16, kind="ExternalInput")
    if bias_in:
        bvec = nc.dram_tensor("bvec", [c_in, 1], F32, kind="ExternalInput")
    hT = nc.dram_tensor("hT", [M, SH], F16, kind="ExternalOutput")
    if not merged:
        alT = nc.dram_tensor("alT", [m_al, SH], F16, kind="ExternalOutput")

    with tile.TileContext(nc) as tc:
        with (
            tc.tile_pool(name="const", bufs=1) as constp,
            tc.tile_pool(name="xin", bufs=2) as xinp,
            tc.tile_pool(name="hout", bufs=2) as houtp,
            tc.tile_pool(name="work", bufs=4) as workp,
            tc.tile_pool(name="psH", bufs=4, space="PSUM") as psH,
            tc.tile_pool(name="psA", bufs=4, space="PSUM") as psA,
        ):
            w_sb = constp.tile([c_in, M], F16)
            nc.scalar.dma_start(out=w_sb[:], in_=w[:])
            if not merged:
                wal_sb = constp.tile([c_in, m_al], F16)
                nc.scalar.dma_start(out=wal_sb[:], in_=wal[:])
            if bias_in:
                b_sb = constp.tile([c_in, 1], F32)
                nc.scalar.dma_start(out=b_sb[:], in_=bvec[:])

            def body(_iv=None):
                xq = [xinp.tile([c_in, QS], F16, tag=f"x{q}", name=f"xq{q}")
                      for q in range(QN)]
                for q in range(QN):
                    nc.sync.dma_start(out=xq[q][:],
                                      in_=xT[:, q * QS:(q + 1) * QS])
                hout = houtp.tile([M, SH], F16, tag="h")
                if not merged:
                    alout = houtp.tile([m_al, SH], F16, tag="al")
                for c0 in range(0, SH, CH):
                    q, qo = c0 // QS, c0 % QS
                    rhs = xq[q][:, qo:qo + CH]
                    if elu:
                        if bias_in:
                            nc.vector.tensor_scalar(
                                rhs, rhs, b_sb[:, 0:1], None, OP.add)
                        # elu(x) = max(x,0) - 1 + exp(min(x,0))
                        mn = workp.tile([c_in, CH], F16, tag="mn")
                        nc.vector.tensor_scalar(
                            mn[:], rhs, 0.0, None, OP.min)
                        nc.scalar.activation(mn[:], mn[:], AF.Exp)
                        mx = workp.tile([c_in, CH], F16, tag="mx")
                        nc.vector.tensor_scalar(
                            mx[:], rhs, 0.0, -1.0, OP.max, OP.add)
                        xe = workp.tile([c_in, CH], F16, tag="xe")
                        nc.vector.tensor_tensor(
                            out=xe[:], in0=mx[:], in1=mn[:], op=OP.add)
                        rhs = xe[:]
                    ph = psH.tile([M, CH], F32, tag="ph")
                    nc.tensor.matmul(ph[:], w_sb[:], rhs,
                                     start=True, stop=True)
                    nc.scalar.activation(hout[:, c0:c0 + CH], ph[:], AF.Copy)
                    if not merged:
                        pa = psA.tile([m_al, CH], F32, tag="pa")
                        nc.tensor.matmul(pa[:], wal_sb[:], rhs,
                                         start=True, stop=True)
                        nc.vector.tensor_copy(alout[:, c0:c0 + CH], pa[:])
                for hhalf in range(2):
                    h0 = hhalf * (SH // 2)
                    nc.scalar.dma_start(out=hT[:, h0:h0 + SH // 2],
                                        in_=hout[:, h0:h0 + SH // 2])
                if not merged:
                    nc.scalar.dma_start(out=alT[:], in_=alout[:])

            if bench_loop > 1:
                with tc.For_i(0, bench_loop, 1) as _iv:
                    body(_iv)
            else:
                body()
    _finalize_kernel(nc)
    return nc


def _build_edge_g(D_list, groups, TOT, Cc, H, bias_out=False, bench_loop=1):
    """Edge aggregation over degree-sorted grids.  Per group of windows:
    one h[src] grid DMA, one DVE logit add per window, one ACT leaky-relu,
    one ACT exp into the message tile's trailing EB columns, one DVE
    multiply, then D accumulating identity matmuls per window.  Epilogues
    run one group late so no engine stalls on PSUM completion."""
    EB = 8
    SLOT = Cc + EB
    G = Cc // EB
    NW = len(D_list)
    GS = max(sd for _, _, _, sd in groups)
    NWmax = max(nw for _, nw, _, _ in groups)

    nc = bass.Bass()
    hsrc = nc.dram_tensor("hsrc", [P, TOT * Cc], F16, kind="ExternalInput")
    als = nc.dram_tensor("als", [P, TOT * H], F16, kind="ExternalInput")
    ald = nc.dram_tensor("ald", [P, NW * H], F16, kind="ExternalInput")
    ident = nc.dram_tensor("ident", [P, P], F8, kind="ExternalInput")
    if bias_out:
        brep = nc.dram_tensor("brep", [P, Cc], F32, kind="ExternalInput")
    out = nc.dram_tensor("out", [NW * P, Cc], F16, kind="ExternalOutput")

    with tile.TileContext(nc) as tc:
        with (
            tc.tile_pool(name="const", bufs=1) as constp,
            tc.tile_pool(name="alsall", bufs=2) as alsp,
            tc.tile_pool(name="hs", bufs=2) as hsp,
            tc.tile_pool(name="za", bufs=2) as zap,
            tc.tile_pool(name="msg", bufs=2) as msgp,
            tc.tile_pool(name="epi", bufs=4) as epip,
            tc.tile_pool(name="og", bufs=2) as ogp,
            tc.tile_pool(name="psW", bufs=6, space="PSUM") as pswp,
        ):
            BSLOT = 512 // SLOT      # windows per PSUM bank
            ident_sb = constp.tile([P, P], F8)
            nc.scalar.dma_start(out=ident_sb[:], in_=ident[:])
            ebias_sb = constp.tile([P, 1], F32)
            nc.vector.memset(ebias_sb[:], EXP_BIAS)
            if bias_out:
                brep_sb = constp.tile([P, Cc], F32)
                nc.scalar.dma_start(out=brep_sb[:], in_=brep[:])

            pend = []

            def main(grp, als_sb, ald_sb):
                i0, nw, off0, sd = grp
                hs = hsp.tile([P, GS * Cc], F16, tag="hs")
                nc.sync.dma_start(out=hs[:, :sd * Cc],
                                  in_=hsrc[:, off0 * Cc:(off0 + sd) * Cc])
                za = zap.tile([P, GS * H], F16, tag="za")
                doff = 0
                for wl in range(nw):
                    D = int(D_list[i0 + wl])
                    o0 = (off0 + doff) * H
                    if H > 1:
                        av = als_sb[:, o0:o0 + D * H].rearrange(
                            "p (d h) -> p d h", d=D)
                        zv = za[:, doff * H:(doff + D) * H].rearrange(
                            "p (d h) -> p d h", d=D)
                        ad = ald_sb[:, (i0 + wl) * H:(i0 + wl + 1) * H]
                        ab = bass.AP(ad.tensor, ad.offset,
                                     [ad.ap[0], [0, D], [1, H]])
                    else:
                        av = als_sb[:, o0:o0 + D]
                        zv = za[:, doff:doff + D]
                        ad = ald_sb[:, i0 + wl:i0 + wl + 1]
                        ab = bass.AP(ad.tensor, ad.offset,
                                     [ad.ap[0], [0, D]])
                    nc.vector.tensor_tensor(out=zv, in0=av, in1=ab, op=OP.add)
                    doff += D
                nc.scalar.activation(za[:, :sd * H], za[:, :sd * H],
                                     AF.Prelu, alpha=NEG_SLOPE)
                msg = msgp.tile([P, GS * SLOT], F16, tag="msg")
                m3 = msg[:, :sd * SLOT].rearrange("p (d s) -> p d s", s=SLOT)
                eb_out = m3[:, :, Cc:Cc + EB]
                if H > 1:
                    e_in = za[:, :sd * H].rearrange("p (d h) -> p d h", d=sd)
                else:
                    z0 = za[:, :sd]
                    e_in = bass.AP(z0.tensor, z0.offset,
                                   [z0.ap[0], [1, sd], [0, EB]])
                nc.scalar.activation(eb_out, e_in, AF.Exp, bias=ebias_sb[:])
                mo = m3[:, :, 0:Cc].rearrange("p d (g h) -> p d g h", h=EB)
                hi = hs[:, :sd * Cc].rearrange(
                    "p (d g h) -> p d g h", d=sd, h=EB)
                e0 = eb_out
                ei = bass.AP(e0.tensor, e0.offset,
                             [e0.ap[0], e0.ap[1], [0, G], [1, EB]])
                nc.vector.tensor_tensor(out=mo, in0=hi, in1=ei, op=OP.mult)
                doff = 0
                bank = None
                for wl in range(nw):
                    D = int(D_list[i0 + wl])
                    if wl % BSLOT == 0:
                        bank = pswp.tile([P, 512], F32, tag="psw",
                                         name="pswbank")
                    sl = (wl % BSLOT) * SLOT
                    psw = bank[:, sl:sl + SLOT]
                    for j in range(D):
                        mv = msg[:, (doff + j) * SLOT:(doff + j + 1) * SLOT]
                        nc.tensor.matmul(psw, ident_sb[:], mv,
                                         start=(j == 0), stop=(j == D - 1))
                    pend.append(psw)
                    doff += D

            def epilogue(grp):
                i0, nw, off0, sd = grp
                og = ogp.tile([P, NWmax * Cc], F16, tag="og")
                for wl in range(nw):
                    psw = pend.pop(0)
                    den = epip.tile([P, EB], F32, tag="den")
                    nc.scalar.activation(den[:], psw[:, Cc:Cc + EB],
                                         AF.Copy, bias=EPS)
                    rec = epip.tile([P, EB], F16, tag="rec")
                    with nc.allow_low_precision(
                            reason="softmax denominators are O(1)"):
                        nc.vector.reciprocal(rec[:], den[:])
                    o1p = epip.tile([P, Cc], F16, tag="o1p")
                    nc.scalar.activation(o1p[:], psw[:, 0:Cc], AF.Copy)
                    r_ap = rec[:]
                    r_b = bass.AP(r_ap.tensor, r_ap.offset,
                                  [r_ap.ap[0], [0, G], [1, EB]])
                    o_sl = og[:, wl * Cc:(wl + 1) * Cc]
                    nc.vector.tensor_tensor(
                        out=o_sl.rearrange("p (g h) -> p g h", h=EB),
                        in0=o1p[:].rearrange("p (g h) -> p g h", h=EB),
                        in1=r_b, op=OP.mult)
                    if bias_out:
                        nc.vector.tensor_tensor(
                            out=o_sl, in0=o_sl, in1=brep_sb[:], op=OP.add)
                dr = out[i0 * P:(i0 + nw) * P, :].rearrange(
                    "(w e) c -> e w c", e=P)
                nc.scalar.dma_start(
                    out=dr,
                    in_=og[:, :nw * Cc].rearrange("p (w c) -> p w c", w=nw))

            def body(_iv=None):
                als_sb = alsp.tile([P, TOT * H], F16, tag="als")
                nc.sync.dma_start(out=als_sb[:], in_=als[:])
                ald_sb = alsp.tile([P, NW * H], F16, tag="ald")
                nc.scalar.dma_start(out=ald_sb[:], in_=ald[:])
                pend.clear()
                prev = None
                for grp in groups:
                    main(grp, als_sb, ald_sb)
                    if prev is not None:
                        epilogue(prev)
                    prev = grp
                epilogue(prev)

            if bench_loop > 1:
                with tc.For_i(0, bench_loop, 1) as _iv:
                    body(_iv)
            else:
                body()
    _finalize_kernel(nc)
    return nc


# ------------------------------------------------------------------ runner

def _fold_att(W, a):
    heads, hid = a.shape
    return np.einsum("ihc,hc->ih", W.reshape(W.shape[0], heads, hid), a)


class _GatRunner:
    def __init__(self, n_cores=N_CORES):
        self.C = n_cores
        self._graph = None
        self._graph_key = None
        self._kernels = {}
        self.last_maps = {}

    def graph(self, edge_index, n_nodes):
        key = hash(np.asarray(edge_index).tobytes())
        if key != self._graph_key:
            self._graph = _Graph(edge_index, n_nodes, self.C)
            self._graph_key = key
            self._kernels.clear()
        return self._graph

    def kernel(self, name, bench_loop=1, **kw):
        key = (name, bench_loop, tuple(sorted(kw.items())))
        if key not in self._kernels:
            g = self._graph
            if name.startswith("P"):
                self._kernels[key] = _build_node(
                    g.shard_nodes, bench_loop=bench_loop, **kw)
            elif name == "E1":
                self._kernels[key] = _build_edge_g(
                    g.D, g.groups, g.TOT, 128, 8, bench_loop=bench_loop)
            else:
                self._kernels[key] = _build_edge_g(
                    g.D, g.groups, g.TOT, 64, 1, bench_loop=bench_loop, **kw)
        return self._kernels[key]

    def _run(self, name, nc, maps):
        self.last_maps[name] = maps
        res = run_bass_kernel_spmd(nc, maps, core_ids=list(range(self.C)))
        return res.results

    def run(self, x, edge_index, W1, a_src1, a_dst1, b1, W2, a_src2, a_dst2,
            b2):
        C = self.C
        N, IN_C = x.shape
        HEADS, HID = a_src1.shape
        HC = HEADS * HID
        OUT_C = W2.shape[1]
        g = self.graph(edge_index, N)
        SH = g.shard_nodes
        # (c,h)-interleaved channel order for the layer-1 hidden features:
        # col c*H+h of h1 holds math channel h*HID+c. Folded into W1's
        # columns (P0) and W2's rows (P2) on the host - pure permutation.
        perm = np.array([(j % HEADS) * HID + j // HEADS
                         for j in range(HC)], dtype=np.int64)

        # ---- P0: per-node h1 / logits --------------------------------
        xT_pad = np.zeros((IN_C, g.n_pad), dtype=np.float16)
        xT_pad[:, :N] = np.asarray(x, np.float32).T
        w1 = np.asarray(W1, np.float32)
        wal1 = np.concatenate(
            [_fold_att(w1, np.asarray(a_src1, np.float32)),
             _fold_att(w1, np.asarray(a_dst1, np.float32))], axis=1)
        mapsP0 = [{"xT": np.ascontiguousarray(xT_pad[:, k * SH:(k + 1) * SH]),
                   "w": np.ascontiguousarray(w1[:, perm]).astype(np.float16),
                   "wal": wal1.astype(np.float16)} for k in range(C)]
        ncP0 = self.kernel("P0", c_in=IN_C, m_h=HC, m_al=2 * HEADS,
                           elu=False, bias_in=False)
        resP0 = self._run("P0", ncP0, mapsP0)
        h1 = np.ascontiguousarray(
            np.concatenate([r["hT"] for r in resP0], axis=1).T)[:N]
        al1 = np.concatenate([r["alT"] for r in resP0], axis=1)  # [16, Np]
        als1 = np.ascontiguousarray(al1[:HEADS, :N].T)
        ald1 = np.ascontiguousarray(al1[HEADS:, :N].T)

        # ---- E1: layer-1 edge aggregation ----------------------------
        id8 = g.ident8()
        mapsE1 = [{"hsrc": g.stream_h(h1, k),
                   "als": g.stream_als(als1, k),
                   "ald": g.stream_ald(ald1, k),
                   "ident": id8} for k in range(C)]
        ncE1 = self.kernel("E1")
        resE1 = self._run("E1", ncE1, mapsE1)
        out1 = np.concatenate([r["out"] for r in resE1], axis=0)
        # rows of out1 are (core, slot, row) -> natural node rowmap
        rowmap = g.rows_nodes.reshape(-1)            # [C*wpc*P]

        # ---- P2: ELU + per-node h2 / logits --------------------------
        o1T = np.ascontiguousarray(out1.T)           # [HC, C*SH] f16
        w2 = np.asarray(W2, np.float32)
        wal2 = np.concatenate(
            [_fold_att(w2, np.asarray(a_src2, np.float32)),
             _fold_att(w2, np.asarray(a_dst2, np.float32))], axis=1)
        b1nz = bool(np.any(np.asarray(b1)))
        w2all = np.concatenate([w2[perm], wal2[perm]], axis=1)  # [HC, 66]
        mapsP2 = []
        for k in range(C):
            m = {"xT": np.ascontiguousarray(o1T[:, k * SH:(k + 1) * SH]),
                 "w": w2all.astype(np.float16)}
            if b1nz:
                m["bvec"] = np.asarray(b1, np.float32)[perm].reshape(HC, 1)
            mapsP2.append(m)
        ncP2 = self.kernel("P2", c_in=HC, m_h=OUT_C, m_al=2, elu=True,
                           bias_in=b1nz)
        resP2 = self._run("P2", ncP2, mapsP2)
        h2al = np.concatenate([r["hT"] for r in resP2], axis=1)  # [66, Np]
        valid = rowmap >= 0
        vrows = rowmap[valid]
        h2 = np.zeros((N, OUT_C), dtype=np.float16)
        h2[vrows] = h2al[:OUT_C].T[valid]
        als2 = np.zeros((N, 1), dtype=np.float16)
        als2[vrows, 0] = h2al[OUT_C][valid]
        ald2 = np.zeros((N, 1), dtype=np.float16)
        ald2[vrows, 0] = h2al[OUT_C + 1][valid]

        # ---- E2: layer-2 edge aggregation ----------------------------
        b2nz = bool(np.any(np.asarray(b2)))
        mapsE2 = []
        for k in range(C):
            m = {"hsrc": g.stream_h(h2, k),
                 "als": g.stream_als(als2, k),
                 "ald": g.stream_ald(ald2, k),
                 "ident": id8}
            if b2nz:
                m["brep"] = np.tile(np.asarray(b2, np.float32), (P, 1))
            mapsE2.append(m)
        ncE2 = self.kernel("E2", bias_out=b2nz)
        resE2 = self._run("E2", ncE2, mapsE2)
        out2 = np.concatenate([r["out"] for r in resE2], axis=0)
        out_full = np.zeros((N, OUT_C), dtype=np.float32)
        out_full[vrows] = out2[valid]
        return out_full


_RUNNER = _GatRunner()


def kernel(x, edge_index, W1, a_src1, a_dst1, b1, W2, a_src2, a_dst2, b2):
    """Full-input / full-output entry point. Returns [N, OUT_C] float32."""
    args = [np.asarray(v) for v in
            (x, edge_index, W1, a_src1, a_dst1, b1, W2, a_src2, a_dst2, b2)]
    return _RUNNER.run(*args).astype(np.float32)


# revision 8
# speedup vs baseline: 1.5896x; 1.0785x over previous
"""Trainium (trn2) Bass kernel for a 2-layer GAT over N=100k nodes / E=1.7M edges.

Strategy (degree-sorted edge grids + identity-stationary PE accumulation)
-------------------------------------------------------------------------
Nodes are sorted by in-degree on the host and packed into windows of 128
similar-degree destination nodes; windows are dealt round-robin across the 8
NeuronCores.  Each window's edges form a dense grid [128 nodes x D slots]
(D = max in-window degree, padded slots carry -inf logits so exp()==0), so
slot j of all 128 nodes is a 128-edge tile whose destination map is the
IDENTITY: the tensor engine accumulates the per-slot message tiles straight
into the window's PSUM bank with a never-changing fp8 identity stationary.
Degree sorting keeps grid padding at ~1.3%, and the one-hot selection stream
of the classic dst-sorted formulation (128 B/edge of pure index overhead)
disappears entirely.

Each GAT layer runs as TWO SPMD kernels with host-side index gathers (pure
permutations / casts - no host FLOPs) between them:

* node kernel (P0/P2): h = x @ W plus folded attention logits computed once
  per node (dense matmuls).  The full per-core input/output panels live in
  SBUF, loaded/stored with a handful of fat DMAs (the previous per-chunk
  1 KB/partition DMAs were latency-bound at ~140 GB/s).
* edge kernel (E1/E2): streams h[src] grids (256/128 B per edge slot) and
  al_src logit grids (16/2 B); al_dst is a tiny per-window constant.  Windows
  are processed in groups (sum of D <= 96) so DVE/ACT run one fat instruction
  per group: DVE adds the logits, ACT applies leaky-relu then writes
  exp(z-4) into the message tile's trailing 8 columns ((c,h)-interleaved
  broadcast for the 8 heads of layer 1, an 8x replica for layer 2's single
  head so the DVE multiply keeps its packed-innermost 2x mode), DVE scales
  the h grid by the exp block, and PE accumulates [msg | exp] per slot.
  Epilogues (PSUM read, reciprocal, scale, output DMA batched per group) are
  emitted one group LATE so the PE/DVE/ACT pipelines never stall on them.
  Streams ride the SP DMA queue; outputs + constants ride the ACT queue.

Measured per-core DMA floor is ~343 GB/s (HBM fair share); the edge kernels
stream ~58.5 MB (E1) / ~28 MB (E2) per core per inference.

Environment workarounds: this container's walrus build allows only ONE
semaphore wait per instruction (split onto nop carriers post-scheduling), and
the GPSIMD ucode libraries are absent (so no dma_gather/indirect-DMA fast
paths - hence the host-gather design).
"""
import numpy as np

import concourse.bass as bass
import concourse.mybir as mybir
import concourse.tile as tile
from concourse.bass_utils import run_bass_kernel_spmd

P = 128
F16 = mybir.dt.float16
F32 = mybir.dt.float32
F8 = mybir.dt.float8e4
AF = mybir.ActivationFunctionType
OP = mybir.AluOpType
NEG_SLOPE = 0.2
EXP_BIAS = -4.0     # exp(z + EXP_BIAS): constant shift cancels in softmax
NEG_INF = -60000.0  # pad-slot logit: exp(lrelu(.)+bias) underflows to 0
N_CORES = 8
EPS = 1e-30
CH = 448            # node-kernel matmul chunk (PSUM: 448*4B <= 2KB bank)
GCAP = 96           # edge-kernel group capacity (sum of window D's)
NWG = 12            # max windows per group

# ------------------------------------------------------------------ patches

_wsplit_counter = [0]


def _split_excess_waits(nc, max_waits=1):
    """This walrus build rejects >1 sem-wait per instruction ("Too many sync
    wait commands"). Move overflow waits onto same-engine nop carriers."""
    n_split = 0
    for f in nc.m.functions:
        for blk in f.blocks:
            changed = False
            out = []
            for inst in blk.instructions:
                si = inst.sync_info
                if si is not None and len(si.on_wait) > max_waits:
                    waits = list(si.on_wait)
                    keep = waits[len(waits) - max_waits:]
                    overflow = waits[: len(waits) - max_waits]
                    for i in range(0, len(overflow), max_waits):
                        _wsplit_counter[0] += 1
                        nop = mybir.InstNoOp(
                            name=f"I-wsplit-{_wsplit_counter[0]}", ins=[], outs=[])
                        nop.engine = inst.engine
                        nop.sync_info = mybir.SyncInfo(
                            on_wait=overflow[i: i + max_waits], on_update=[])
                        out.append(nop)
                    inst.sync_info = mybir.SyncInfo(
                        on_wait=keep, on_update=list(si.on_update))
                    changed = True
                    n_split += 1
                out.append(inst)
            if changed:
                blk.instructions = out
    return n_split


def _finalize_kernel(nc):
    import bass_rust as _bass_rust
    from concourse.library_config import all_libraries, standard
    from concourse.library_overlay import lower_extended_insts

    inst_type_to_lib_mask = {}
    for lib in all_libraries:
        for inst_type in lib.instructions:
            inst_type_to_lib_mask[inst_type] = inst_type_to_lib_mask.get(
                inst_type, 0) | (1 << lib.index)
    _bass_rust.insert_library_loads(
        nc, inst_type_to_lib_mask, len(all_libraries), standard.index)
    lower_extended_insts(nc)
    _split_excess_waits(nc)


# ------------------------------------------------------------------ host prep

class _Graph:
    """Degree-sorted grid preprocessing: sort nodes by in-degree, pack 128
    similar-degree nodes per window, deal windows round-robin across cores
    (slot i of every core shares one padded depth D_i so all cores run one
    identical SPMD program), and scatter each node's edges into its grid row.
    """

    def __init__(self, edge_index, n_nodes, n_cores):
        self.N = n_nodes
        self.C = n_cores
        src = np.asarray(edge_index[0], dtype=np.int64)
        dst = np.asarray(edge_index[1], dtype=np.int64)
        E = src.shape[0]

        deg = np.bincount(dst, minlength=n_nodes)
        order = np.argsort(deg, kind="stable")

        n_win_total = (n_nodes + P - 1) // P
        self.wpc = (n_win_total + n_cores - 1) // n_cores
        n_win = self.wpc * n_cores
        self.n_pad = n_win * P
        self.shard_nodes = self.wpc * P
        n_dummy = self.n_pad - n_nodes

        snode = np.full(self.n_pad, -1, dtype=np.int64)
        snode[n_dummy:] = order                      # ascending degree
        # rows_nodes[k][i, e] = natural node id at (core k, slot i, row e)
        self.rows_nodes = np.ascontiguousarray(
            snode.reshape(self.wpc, n_cores, P).transpose(1, 0, 2))

        wdeg = np.where(snode >= 0, deg[np.clip(snode, 0, None)], 0)
        wmax = wdeg.reshape(self.wpc, n_cores, P).max(axis=2)   # [wpc, cores]
        self.D = np.maximum(wmax.max(axis=1), 1).astype(np.int64)  # [wpc]
        self.off = np.concatenate([[0], np.cumsum(self.D)])
        self.TOT = int(self.D.sum())

        # position of each node in the sorted layout
        posq = np.empty(n_nodes, dtype=np.int64)
        posq[order] = np.arange(n_nodes) + n_dummy

        # scatter edges (dst-sorted, ranked within dst run) into grids
        perm = np.argsort(dst, kind="stable")
        src_s = src[perm]
        dst_s = dst[perm]
        bounds = np.searchsorted(dst_s, np.arange(n_nodes + 1))
        j_e = np.arange(E) - bounds[dst_s]           # rank within dst run
        q_e = posq[dst_s]
        g_e = q_e // P
        row_e = q_e % P
        core_e = g_e % n_cores
        slot_e = g_e // n_cores
        flat_e = self.off[slot_e] + j_e              # grid slot within [TOT]
        self.gidx = np.zeros((n_cores, self.TOT, P), dtype=np.int32)
        self.gidx[core_e, flat_e, row_e] = (src_s + 1).astype(np.int32)

        # group windows: sum(D) <= GCAP, <= NWG windows
        groups = []
        i = 0
        while i < self.wpc:
            i0, sd, nw = i, 0, 0
            while (i < self.wpc and nw < NWG
                   and (nw == 0 or sd + int(self.D[i]) <= GCAP)):
                sd += int(self.D[i])
                i += 1
                nw += 1
            groups.append((i0, nw, int(self.off[i0]), sd))
        self.groups = groups
        self.D_key = tuple(int(d) for d in self.D)

    def stream_h(self, table, core):
        """[128, TOT*C] f16 grid gather: table rows by gidx (0 = zero pad)."""
        C = table.shape[1]
        tp = np.zeros((self.N + 1, C), dtype=np.float16)
        tp[1:] = table
        arr = tp[self.gidx[core]]                    # [TOT, P, C]
        return np.ascontiguousarray(arr.transpose(1, 0, 2)).reshape(
            P, self.TOT * C)

    def stream_als(self, table, core):
        """[128, TOT*H] f16: al_src grid; pad slots -> NEG_INF so exp()==0.
        Dummy rows get one j=0 slot with logit 0 so their softmax denominator
        stays finite (their h rows are zero, so the output row is 0)."""
        H = table.shape[1]
        tp = np.full((self.N + 1, H), NEG_INF, dtype=np.float16)
        tp[1:] = table
        arr = tp[self.gidx[core]]                    # [TOT, P, H]
        i_d, e_d = np.nonzero(self.rows_nodes[core] < 0)
        arr[self.off[i_d], e_d, :] = 0.0
        return np.ascontiguousarray(arr.transpose(1, 0, 2)).reshape(
            P, self.TOT * H)

    def stream_ald(self, table, core):
        """[128, wpc*H] f16: al_dst per (window, row). Dummy rows -> 0."""
        H = table.shape[1]
        tp = np.zeros((self.N + 1, H), dtype=np.float16)
        tp[1:] = table
        arr = tp[self.rows_nodes[core] + 1]          # [wpc, P, H]
        return np.ascontiguousarray(arr.transpose(1, 0, 2)).reshape(
            P, self.wpc * H)

    def ident8(self):
        import ml_dtypes
        return np.eye(P, dtype=np.float32).astype(ml_dtypes.float8_e4m3)


# ------------------------------------------------------------------ builders

def _build_node(SH, c_in, m_h, m_al, elu, bias_in, bench_loop=1):
    """Per-node transform: hT = (elu?(xT+b)) @ w, alT = same @ wal.
    When m_h+m_al <= 128 the two matmuls merge into one.  The whole per-core
    panel is SBUF-resident: quarters stream in with fat DMAs, chunked matmuls
    write a staged output panel, and a few fat DMAs store it."""
    merged = (m_h + m_al) <= P
    M = m_h + m_al if merged else m_h
    QN = 4
    QS = SH // QN
    assert SH % QN == 0 and QS % CH == 0
    nc = bass.Bass()
    xT = nc.dram_tensor("xT", [c_in, SH], F16, kind="ExternalInput")
    w = nc.dram_tensor("w", [c_in, M], F16, kind="ExternalInput")
    if not merged:
        wal = nc.dram_tensor("wal", [c_in, m_al], F16, kind="ExternalInput")
    if bias_in:
        bvec = nc.dram_tensor("bvec", [c_in, 1], F32, kind="ExternalInput")
    hT = nc.dram_tensor("hT", [M, SH], F16, kind="ExternalOutput")
    if not merged:
        alT = nc.dram_tensor("alT", [m_al, SH], F16, kind="ExternalOutput")

    with tile.TileContext(nc) as tc:
        with (
            tc.tile_pool(name="const", bufs=1) as constp,
            tc.tile_pool(name="xin", bufs=2) as xinp,
            tc.tile_pool(name="hout", bufs=2) as houtp,
            tc.tile_pool(name="work", bufs=4) as workp,
            tc.tile_pool(name="psH", bufs=4, space="PSUM") as psH,
            tc.tile_pool(name="psA", bufs=4, space="PSUM") as psA,
        ):
            w_sb = constp.tile([c_in, M], F16)
            nc.scalar.dma_start(out=w_sb[:], in_=w[:])
            if not merged:
                wal_sb = constp.tile([c_in, m_al], F16)
                nc.scalar.dma_start(out=wal_sb[:], in_=wal[:])
            if bias_in:
                b_sb = constp.tile([c_in, 1], F32)
                nc.scalar.dma_start(out=b_sb[:], in_=bvec[:])

            def body(_iv=None):
                xq = [xinp.tile([c_in, QS], F16, tag=f"x{q}", name=f"xq{q}")
                      for q in range(QN)]
                for q in range(QN):
                    eng = nc.sync if q % 2 == 0 else nc.scalar
                    eng.dma_start(out=xq[q][:],
                                  in_=xT[:, q * QS:(q + 1) * QS])
                hout = houtp.tile([M, SH], F16, tag="h")
                if not merged:
                    alout = houtp.tile([m_al, SH], F16, tag="al")
                for ci, c0 in enumerate(range(0, SH, CH)):
                    q, qo = c0 // QS, c0 % QS
                    rhs = xq[q][:, qo:qo + CH]
                    if elu:
                        if bias_in:
                            nc.vector.tensor_scalar(
                                rhs, rhs, b_sb[:, 0:1], None, OP.add)
                        # elu(x) = max(x,0) + (min(exp(x),1) - 1); exp reads
                        # the DMA'd chunk directly so ACT never waits on DVE
                        et = workp.tile([c_in, CH], F16, tag="et")
                        nc.scalar.activation(et[:], rhs, AF.Exp)
                        nc.vector.tensor_scalar(
                            et[:], et[:], 1.0, -1.0, OP.min, OP.add)
                        xe = workp.tile([c_in, CH], F16, tag="xe")
                        nc.vector.scalar_tensor_tensor(
                            xe[:], rhs, 0.0, et[:], OP.max, OP.add)
                        rhs = xe[:]
                    ph = psH.tile([M, CH], F32, tag="ph")
                    nc.tensor.matmul(ph[:], w_sb[:], rhs,
                                     start=True, stop=True)
                    if elu and ci % 2 == 1:
                        nc.vector.tensor_copy(hout[:, c0:c0 + CH], ph[:])
                    else:
                        nc.scalar.activation(hout[:, c0:c0 + CH], ph[:],
                                             AF.Copy)
                    if not merged:
                        pa = psA.tile([m_al, CH], F32, tag="pa")
                        nc.tensor.matmul(pa[:], wal_sb[:], rhs,
                                         start=True, stop=True)
                        nc.vector.tensor_copy(alout[:, c0:c0 + CH], pa[:])
                for hhalf in range(2):
                    h0 = hhalf * (SH // 2)
                    nc.scalar.dma_start(out=hT[:, h0:h0 + SH // 2],
                                        in_=hout[:, h0:h0 + SH // 2])
                if not merged:
                    nc.scalar.dma_start(out=alT[:], in_=alout[:])

            if bench_loop > 1:
                with tc.For_i(0, bench_loop, 1) as _iv:
                    body(_iv)
            else:
                body()
    _finalize_kernel(nc)
    return nc


def _build_edge_g(D_list, groups, TOT, Cc, H, bias_out=False, bench_loop=1):
    """Edge aggregation over degree-sorted grids.  Per group of windows:
    one h[src] grid DMA, one DVE logit add per window, one ACT leaky-relu,
    one ACT exp into the message tile's trailing EB columns, one DVE
    multiply, then D accumulating identity matmuls per window.  Epilogues
    run one group late so no engine stalls on PSUM completion."""
    EB = 8
    SLOT = Cc + EB
    G = Cc // EB
    NW = len(D_list)
    GS = max(sd for _, _, _, sd in groups)
    NWmax = max(nw for _, nw, _, _ in groups)

    nc = bass.Bass()
    hsrc = nc.dram_tensor("hsrc", [P, TOT * Cc], F16, kind="ExternalInput")
    als = nc.dram_tensor("als", [P, TOT * H], F16, kind="ExternalInput")
    ald = nc.dram_tensor("ald", [P, NW * H], F16, kind="ExternalInput")
    ident = nc.dram_tensor("ident", [P, P], F8, kind="ExternalInput")
    if bias_out:
        brep = nc.dram_tensor("brep", [P, Cc], F32, kind="ExternalInput")
    out = nc.dram_tensor("out", [NW * P, Cc], F16, kind="ExternalOutput")

    with tile.TileContext(nc) as tc:
        with (
            tc.tile_pool(name="const", bufs=1) as constp,
            tc.tile_pool(name="aldp", bufs=2) as aldp,
            tc.tile_pool(name="alg", bufs=3) as algp,
            tc.tile_pool(name="hs", bufs=3) as hsp,
            tc.tile_pool(name="za", bufs=2) as zap,
            tc.tile_pool(name="msg", bufs=3) as msgp,
            tc.tile_pool(name="epi", bufs=3) as epip,
            tc.tile_pool(name="og", bufs=2) as ogp,
            tc.tile_pool(name="psW", bufs=6, space="PSUM") as pswp,
        ):
            BSLOT = 512 // SLOT      # windows per PSUM bank
            ident_sb = constp.tile([P, P], F8)
            nc.scalar.dma_start(out=ident_sb[:], in_=ident[:])
            ebias_sb = constp.tile([P, 1], F32)
            nc.vector.memset(ebias_sb[:], EXP_BIAS)
            if bias_out:
                brep_sb = constp.tile([P, Cc], F32)
                nc.scalar.dma_start(out=brep_sb[:], in_=brep[:])

            pend = []

            def front(grp, ald_sb):
                """DMA + logit add + leaky-relu + exp for one group."""
                i0, nw, off0, sd = grp
                hs = hsp.tile([P, GS * Cc], F16, tag="hs")
                nc.sync.dma_start(out=hs[:, :sd * Cc],
                                  in_=hsrc[:, off0 * Cc:(off0 + sd) * Cc])
                alg = algp.tile([P, GS * H], F16, tag="alg")
                nc.sync.dma_start(out=alg[:, :sd * H],
                                  in_=als[:, off0 * H:(off0 + sd) * H])
                za = zap.tile([P, GS * H], F16, tag="za")
                doff = 0
                for wl in range(nw):
                    D = int(D_list[i0 + wl])
                    o0 = doff * H
                    if H > 1:
                        av = alg[:, o0:o0 + D * H].rearrange(
                            "p (d h) -> p d h", d=D)
                        zv = za[:, o0:o0 + D * H].rearrange(
                            "p (d h) -> p d h", d=D)
                        ad = ald_sb[:, (i0 + wl) * H:(i0 + wl + 1) * H]
                        ab = bass.AP(ad.tensor, ad.offset,
                                     [ad.ap[0], [0, D], [1, H]])
                    else:
                        av = alg[:, o0:o0 + D]
                        zv = za[:, o0:o0 + D]
                        ad = ald_sb[:, i0 + wl:i0 + wl + 1]
                        ab = bass.AP(ad.tensor, ad.offset,
                                     [ad.ap[0], [0, D]])
                    nc.vector.tensor_tensor(out=zv, in0=av, in1=ab, op=OP.add)
                    doff += D
                nc.scalar.activation(za[:, :sd * H], za[:, :sd * H],
                                     AF.Prelu, alpha=NEG_SLOPE)
                msg = msgp.tile([P, GS * SLOT], F16, tag="msg")
                m3 = msg[:, :sd * SLOT].rearrange("p (d s) -> p d s", s=SLOT)
                eb_out = m3[:, :, Cc:Cc + EB]
                if H > 1:
                    e_in = za[:, :sd * H].rearrange("p (d h) -> p d h", d=sd)
                else:
                    z0 = za[:, :sd]
                    e_in = bass.AP(z0.tensor, z0.offset,
                                   [z0.ap[0], [1, sd], [0, EB]])
                nc.scalar.activation(eb_out, e_in, AF.Exp, bias=ebias_sb[:])
                return hs, msg

            def back(grp, st):
                """DVE message multiply + PE identity accumulation."""
                i0, nw, off0, sd = grp
                hs, msg = st
                m3 = msg[:, :sd * SLOT].rearrange("p (d s) -> p d s", s=SLOT)
                eb_out = m3[:, :, Cc:Cc + EB]
                mo = m3[:, :, 0:Cc].rearrange("p d (g h) -> p d g h", h=EB)
                hi = hs[:, :sd * Cc].rearrange(
                    "p (d g h) -> p d g h", d=sd, h=EB)
                ei = bass.AP(eb_out.tensor, eb_out.offset,
                             [eb_out.ap[0], eb_out.ap[1], [0, G], [1, EB]])
                nc.vector.tensor_tensor(out=mo, in0=hi, in1=ei, op=OP.mult)
                doff = 0
                bank = None
                for wl in range(nw):
                    D = int(D_list[i0 + wl])
                    if wl % BSLOT == 0:
                        bank = pswp.tile([P, 512], F32, tag="psw",
                                         name="pswbank")
                    sl = (wl % BSLOT) * SLOT
                    psw = bank[:, sl:sl + SLOT]
                    for j in range(D):
                        mv = msg[:, (doff + j) * SLOT:(doff + j + 1) * SLOT]
                        nc.tensor.matmul(psw, ident_sb[:], mv,
                                         start=(j == 0), stop=(j == D - 1))
                    pend.append(psw)
                    doff += D

            def epilogue(grp):
                """One f16 PSUM copy per window, then a single reciprocal +
                scale + output DMA for the whole group."""
                i0, nw, off0, sd = grp
                op_t = epip.tile([P, NWmax * SLOT], F16, tag="o1p")
                for wl in range(nw):
                    psw = pend.pop(0)
                    nc.scalar.activation(op_t[:, wl * SLOT:(wl + 1) * SLOT],
                                         psw, AF.Copy)
                opv = op_t[:, :nw * SLOT]
                rec = epip.tile([P, NWmax * EB], F16, tag="rec")
                rv = rec[:, :nw * EB].rearrange("p (w h) -> p w h", w=nw)
                dap = bass.AP(opv.tensor, opv.offset + Cc,
                              [opv.ap[0], [SLOT, nw], [1, EB]])
                with nc.allow_low_precision(
                        reason="softmax denominators are O(1)"):
                    nc.vector.reciprocal(rv, dap)
                og = ogp.tile([P, NWmax * Cc], F16, tag="og")
                o_in = bass.AP(opv.tensor, opv.offset,
                               [opv.ap[0], [SLOT, nw], [EB, G], [1, EB]])
                r0 = rec[:]
                r_b = bass.AP(r0.tensor, r0.offset,
                              [r0.ap[0], [EB, nw], [0, G], [1, EB]])
                oo = og[:, :nw * Cc].rearrange(
                    "p (w g h) -> p w g h", w=nw, h=EB)
                nc.vector.tensor_tensor(out=oo, in0=o_in, in1=r_b,
                                        op=OP.mult)
                if bias_out:
                    ov2 = og[:, :nw * Cc].rearrange("p (w c) -> p w c", w=nw)
                    b0 = brep_sb[:]
                    b_b = bass.AP(b0.tensor, b0.offset,
                                  [b0.ap[0], [0, nw], [1, Cc]])
                    nc.vector.tensor_tensor(out=ov2, in0=ov2, in1=b_b,
                                            op=OP.add)
                dr = out[i0 * P:(i0 + nw) * P, :].rearrange(
                    "(w e) c -> e w c", e=P)
                nc.scalar.dma_start(
                    out=dr,
                    in_=og[:, :nw * Cc].rearrange("p (w c) -> p w c", w=nw))

            def body(_iv=None):
                ald_sb = aldp.tile([P, NW * H], F16, tag="ald")
                nc.scalar.dma_start(out=ald_sb[:], in_=ald[:])
                pend.clear()
                sts = [None] * len(groups)
                for gi, grp in enumerate(groups):
                    sts[gi] = front(grp, ald_sb)
                    if gi >= 1:
                        back(groups[gi - 1], sts[gi - 1])
                        sts[gi - 1] = None
                    if gi >= 2:
                        epilogue(groups[gi - 2])
                ng = len(groups)
                back(groups[ng - 1], sts[ng - 1])
                if ng >= 2:
                    epilogue(groups[ng - 2])
                epilogue(groups[ng - 1])

            if bench_loop > 1:
                with tc.For_i(0, bench_loop, 1) as _iv:
                    body(_iv)
            else:
                body()
    _finalize_kernel(nc)
    return nc


# ------------------------------------------------------------------ runner

def _fold_att(W, a):
    heads, hid = a.shape
    return np.einsum("ihc,hc->ih", W.reshape(W.shape[0], heads, hid), a)


class _GatRunner:
    def __init__(self, n_cores=N_CORES):
        self.C = n_cores
        self._graph = None
        self._graph_key = None
        self._kernels = {}
        self.last_maps = {}

    def graph(self, edge_index, n_nodes):
        key = hash(np.asarray(edge_index).tobytes())
        if key != self._graph_key:
            self._graph = _Graph(edge_index, n_nodes, self.C)
            self._graph_key = key
            self._kernels.clear()
        return self._graph

    def kernel(self, name, bench_loop=1, **kw):
        key = (name, bench_loop, tuple(sorted(kw.items())))
        if key not in self._kernels:
            g = self._graph
            if name.startswith("P"):
                self._kernels[key] = _build_node(
                    g.shard_nodes, bench_loop=bench_loop, **kw)
            elif name == "E1":
                self._kernels[key] = _build_edge_g(
                    g.D, g.groups, g.TOT, 128, 8, bench_loop=bench_loop)
            else:
                self._kernels[key] = _build_edge_g(
                    g.D, g.groups, g.TOT, 64, 1, bench_loop=bench_loop, **kw)
        return self._kernels[key]

    def _run(self, name, nc, maps):
        self.last_maps[name] = maps
        res = run_bass_kernel_spmd(nc, maps, core_ids=list(range(self.C)))
        return res.results

    def run(self, x, edge_index, W1, a_src1, a_dst1, b1, W2, a_src2, a_dst2,
            b2):
        C = self.C
        N, IN_C = x.shape
        HEADS, HID = a_src1.shape
        HC = HEADS * HID
        OUT_C = W2.shape[1]
        g = self.graph(edge_index, N)
        SH = g.shard_nodes
        # (c,h)-interleaved channel order for the layer-1 hidden features:
        # col c*H+h of h1 holds math channel h*HID+c. Folded into W1's
        # columns (P0) and W2's rows (P2) on the host - pure permutation.
        perm = np.array([(j % HEADS) * HID + j // HEADS
                         for j in range(HC)], dtype=np.int64)

        # ---- P0: per-node h1 / logits --------------------------------
        xT_pad = np.zeros((IN_C, g.n_pad), dtype=np.float16)
        xT_pad[:, :N] = np.asarray(x, np.float32).T
        w1 = np.asarray(W1, np.float32)
        wal1 = np.concatenate(
            [_fold_att(w1, np.asarray(a_src1, np.float32)),
             _fold_att(w1, np.asarray(a_dst1, np.float32))], axis=1)
        mapsP0 = [{"xT": np.ascontiguousarray(xT_pad[:, k * SH:(k + 1) * SH]),
                   "w": np.ascontiguousarray(w1[:, perm]).astype(np.float16),
                   "wal": wal1.astype(np.float16)} for k in range(C)]
        ncP0 = self.kernel("P0", c_in=IN_C, m_h=HC, m_al=2 * HEADS,
                           elu=False, bias_in=False)
        resP0 = self._run("P0", ncP0, mapsP0)
        h1 = np.ascontiguousarray(
            np.concatenate([r["hT"] for r in resP0], axis=1).T)[:N]
        al1 = np.concatenate([r["alT"] for r in resP0], axis=1)  # [16, Np]
        als1 = np.ascontiguousarray(al1[:HEADS, :N].T)
        ald1 = np.ascontiguousarray(al1[HEADS:, :N].T)

        # ---- E1: layer-1 edge aggregation ----------------------------
        id8 = g.ident8()
        mapsE1 = [{"hsrc": g.stream_h(h1, k),
                   "als": g.stream_als(als1, k),
                   "ald": g.stream_ald(ald1, k),
                   "ident": id8} for k in range(C)]
        ncE1 = self.kernel("E1")
        resE1 = self._run("E1", ncE1, mapsE1)
        out1 = np.concatenate([r["out"] for r in resE1], axis=0)
        # rows of out1 are (core, slot, row) -> natural node rowmap
        rowmap = g.rows_nodes.reshape(-1)            # [C*wpc*P]

        # ---- P2: ELU + per-node h2 / logits --------------------------
        o1T = np.ascontiguousarray(out1.T)           # [HC, C*SH] f16
        w2 = np.asarray(W2, np.float32)
        wal2 = np.concatenate(
            [_fold_att(w2, np.asarray(a_src2, np.float32)),
             _fold_att(w2, np.asarray(a_dst2, np.float32))], axis=1)
        b1nz = bool(np.any(np.asarray(b1)))
        w2all = np.concatenate([w2[perm], wal2[perm]], axis=1)  # [HC, 66]
        mapsP2 = []
        for k in range(C):
            m = {"xT": np.ascontiguousarray(o1T[:, k * SH:(k + 1) * SH]),
                 "w": w2all.astype(np.float16)}
            if b1nz:
                m["bvec"] = np.asarray(b1, np.float32)[perm].reshape(HC, 1)
            mapsP2.append(m)
        ncP2 = self.kernel("P2", c_in=HC, m_h=OUT_C, m_al=2, elu=True,
                           bias_in=b1nz)
        resP2 = self._run("P2", ncP2, mapsP2)
        h2al = np.concatenate([r["hT"] for r in resP2], axis=1)  # [66, Np]
        valid = rowmap >= 0
        vrows = rowmap[valid]
        h2 = np.zeros((N, OUT_C), dtype=np.float16)
        h2[vrows] = h2al[:OUT_C].T[valid]
        als2 = np.zeros((N, 1), dtype=np.float16)
        als2[vrows, 0] = h2al[OUT_C][valid]
        ald2 = np.zeros((N, 1), dtype=np.float16)
        ald2[vrows, 0] = h2al[OUT_C + 1][valid]

        # ---- E2: layer-2 edge aggregation ----------------------------
        b2nz = bool(np.any(np.asarray(b2)))
        mapsE2 = []
        for k in range(C):
            m = {"hsrc": g.stream_h(h2, k),
                 "als": g.stream_als(als2, k),
                 "ald": g.stream_ald(ald2, k),
                 "ident": id8}
            if b2nz:
                m["brep"] = np.tile(np.asarray(b2, np.float32), (P, 1))
            mapsE2.append(m)
        ncE2 = self.kernel("E2", bias_out=b2nz)
        resE2 = self._run("E2", ncE2, mapsE2)
        out2 = np.concatenate([r["out"] for r in resE2], axis=0)
        out_full = np.zeros((N, OUT_C), dtype=np.float32)
        out_full[vrows] = out2[valid]
        return out_full


_RUNNER = _GatRunner()


def kernel(x, edge_index, W1, a_src1, a_dst1, b1, W2, a_src2, a_dst2, b2):
    """Full-input / full-output entry point. Returns [N, OUT_C] float32."""
    args = [np.asarray(v) for v in
            (x, edge_index, W1, a_src1, a_dst1, b1, W2, a_src2, a_dst2, b2)]
    return _RUNNER.run(*args).astype(np.float32)


# revision 39
# speedup vs baseline: 1.6415x; 1.0326x over previous
"""Trainium (trn2) Bass kernel for a 2-layer GAT over N=100k nodes / E=1.7M edges.

Strategy (degree-sorted edge grids + identity-stationary PE accumulation)
-------------------------------------------------------------------------
Nodes are sorted by in-degree on the host and packed into windows of 128
similar-degree destination nodes; windows are dealt round-robin across the 8
NeuronCores.  Each window's edges form a dense grid [128 nodes x D slots]
(D = max in-window degree, padded slots carry -inf logits so exp()==0), so
slot j of all 128 nodes is a 128-edge tile whose destination map is the
IDENTITY: the tensor engine accumulates the per-slot message tiles straight
into the window's PSUM bank with a never-changing fp8 identity stationary.
Degree sorting keeps grid padding at ~1.3%, and the one-hot selection stream
of the classic dst-sorted formulation (128 B/edge of pure index overhead)
disappears entirely.

Each GAT layer runs as TWO SPMD kernels with host-side index gathers (pure
permutations / casts - no host FLOPs) between them:

* node kernel (P0/P2): h = x @ W plus folded attention logits computed once
  per node (dense matmuls).  The full per-core input/output panels live in
  SBUF, loaded/stored with a handful of fat DMAs (the previous per-chunk
  1 KB/partition DMAs were latency-bound at ~140 GB/s).
* edge kernel (E1/E2): streams h[src] grids (256/128 B per edge slot) and
  al_src logit grids (16/2 B); al_dst is a tiny per-window constant.  Windows
  are processed in groups (sum of D <= 96) so DVE/ACT run one fat instruction
  per group: DVE adds the logits, ACT applies leaky-relu then writes
  exp(z-4) into the message tile's trailing 8 columns ((c,h)-interleaved
  broadcast for the 8 heads of layer 1, an 8x replica for layer 2's single
  head so the DVE multiply keeps its packed-innermost 2x mode), DVE scales
  the h grid by the exp block, and PE accumulates [msg | exp] per slot.
  Epilogues (PSUM read, reciprocal, scale, output DMA batched per group) are
  emitted one group LATE so the PE/DVE/ACT pipelines never stall on them.
  Streams ride the SP DMA queue; outputs + constants ride the ACT queue.

Measured per-core DMA floor is ~343 GB/s (HBM fair share); the edge kernels
stream ~58.5 MB (E1) / ~28 MB (E2) per core per inference.

Environment workarounds: this container's walrus build allows only ONE
semaphore wait per instruction (split onto nop carriers post-scheduling), and
the GPSIMD ucode libraries are absent (so no dma_gather/indirect-DMA fast
paths - hence the host-gather design).
"""
import numpy as np

import concourse.bass as bass
import concourse.mybir as mybir
import concourse.tile as tile
from concourse.bass_utils import run_bass_kernel_spmd

P = 128
F16 = mybir.dt.float16
F32 = mybir.dt.float32
F8 = mybir.dt.float8e4
AF = mybir.ActivationFunctionType
OP = mybir.AluOpType
NEG_SLOPE = 0.2
EXP_BIAS = -4.0     # exp(z + EXP_BIAS): constant shift cancels in softmax
NEG_INF = -60000.0  # pad-slot logit: exp(lrelu(.)+bias) underflows to 0
N_CORES = 8
EPS = 1e-30
CH = 448            # node-kernel matmul chunk (PSUM: 448*4B <= 2KB bank)
GCAP1, NWG1 = 96, 12     # E1 group capacity (sum of D's / max windows)
GCAP2, NWG2 = 224, 26    # E2 group capacity (smaller tiles -> fatter groups)

# ------------------------------------------------------------------ patches

_wsplit_counter = [0]


def _split_excess_waits(nc, max_waits=1):
    """This walrus build rejects >1 sem-wait per instruction ("Too many sync
    wait commands"). Move overflow waits onto same-engine nop carriers."""
    n_split = 0
    for f in nc.m.functions:
        for blk in f.blocks:
            changed = False
            out = []
            for inst in blk.instructions:
                si = inst.sync_info
                if si is not None and len(si.on_wait) > max_waits:
                    waits = list(si.on_wait)
                    keep = waits[len(waits) - max_waits:]
                    overflow = waits[: len(waits) - max_waits]
                    for i in range(0, len(overflow), max_waits):
                        _wsplit_counter[0] += 1
                        nop = mybir.InstNoOp(
                            name=f"I-wsplit-{_wsplit_counter[0]}", ins=[], outs=[])
                        nop.engine = inst.engine
                        nop.sync_info = mybir.SyncInfo(
                            on_wait=overflow[i: i + max_waits], on_update=[])
                        out.append(nop)
                    inst.sync_info = mybir.SyncInfo(
                        on_wait=keep, on_update=list(si.on_update))
                    changed = True
                    n_split += 1
                out.append(inst)
            if changed:
                blk.instructions = out
    return n_split


def _finalize_kernel(nc):
    import bass_rust as _bass_rust
    from concourse.library_config import all_libraries, standard
    from concourse.library_overlay import lower_extended_insts

    inst_type_to_lib_mask = {}
    for lib in all_libraries:
        for inst_type in lib.instructions:
            inst_type_to_lib_mask[inst_type] = inst_type_to_lib_mask.get(
                inst_type, 0) | (1 << lib.index)
    _bass_rust.insert_library_loads(
        nc, inst_type_to_lib_mask, len(all_libraries), standard.index)
    lower_extended_insts(nc)
    _split_excess_waits(nc)


# ------------------------------------------------------------------ host prep

class _Graph:
    """Degree-sorted grid preprocessing: sort nodes by in-degree, pack 128
    similar-degree nodes per window, deal windows round-robin across cores
    (slot i of every core shares one padded depth D_i so all cores run one
    identical SPMD program), and scatter each node's edges into its grid row.
    """

    def __init__(self, edge_index, n_nodes, n_cores):
        self.N = n_nodes
        self.C = n_cores
        src = np.asarray(edge_index[0], dtype=np.int64)
        dst = np.asarray(edge_index[1], dtype=np.int64)
        E = src.shape[0]

        deg = np.bincount(dst, minlength=n_nodes)
        order = np.argsort(deg, kind="stable")

        n_win_total = (n_nodes + P - 1) // P
        self.wpc = (n_win_total + n_cores - 1) // n_cores
        n_win = self.wpc * n_cores
        self.n_pad = n_win * P
        self.shard_nodes = self.wpc * P
        n_dummy = self.n_pad - n_nodes

        snode = np.full(self.n_pad, -1, dtype=np.int64)
        snode[n_dummy:] = order                      # ascending degree
        # rows_nodes[k][i, e] = natural node id at (core k, slot i, row e)
        self.rows_nodes = np.ascontiguousarray(
            snode.reshape(self.wpc, n_cores, P).transpose(1, 0, 2))

        wdeg = np.where(snode >= 0, deg[np.clip(snode, 0, None)], 0)
        wmax = wdeg.reshape(self.wpc, n_cores, P).max(axis=2)   # [wpc, cores]
        self.D = np.maximum(wmax.max(axis=1), 1).astype(np.int64)  # [wpc]
        self.off = np.concatenate([[0], np.cumsum(self.D)])
        self.TOT = int(self.D.sum())

        # position of each node in the sorted layout
        posq = np.empty(n_nodes, dtype=np.int64)
        posq[order] = np.arange(n_nodes) + n_dummy

        # scatter edges (dst-sorted, ranked within dst run) into grids
        perm = np.argsort(dst, kind="stable")
        src_s = src[perm]
        dst_s = dst[perm]
        bounds = np.searchsorted(dst_s, np.arange(n_nodes + 1))
        j_e = np.arange(E) - bounds[dst_s]           # rank within dst run
        q_e = posq[dst_s]
        g_e = q_e // P
        row_e = q_e % P
        core_e = g_e % n_cores
        slot_e = g_e // n_cores
        flat_e = self.off[slot_e] + j_e              # grid slot within [TOT]
        self.gidx = np.zeros((n_cores, self.TOT, P), dtype=np.int32)
        self.gidx[core_e, flat_e, row_e] = (src_s + 1).astype(np.int32)

        self.groups1 = self.make_groups(GCAP1, NWG1)
        self.groups2 = self.make_groups(GCAP2, NWG2)
        self.D_key = tuple(int(d) for d in self.D)

    def make_groups(self, gcap, nwg):
        """Window groups: sum(D) <= gcap, <= nwg windows per group."""
        groups = []
        i = 0
        while i < self.wpc:
            i0, sd, nw = i, 0, 0
            while (i < self.wpc and nw < nwg
                   and (nw == 0 or sd + int(self.D[i]) <= gcap)):
                sd += int(self.D[i])
                i += 1
                nw += 1
            groups.append((i0, nw, int(self.off[i0]), sd))
        return groups

    def stream_h(self, table, core):
        """[128, TOT*C] f16 grid gather: table rows by gidx (0 = zero pad)."""
        C = table.shape[1]
        tp = np.zeros((self.N + 1, C), dtype=np.float16)
        tp[1:] = table
        arr = tp[self.gidx[core]]                    # [TOT, P, C]
        return np.ascontiguousarray(arr.transpose(1, 0, 2)).reshape(
            P, self.TOT * C)

    def stream_als(self, table, core):
        """[128, TOT*H] f16: al_src grid; pad slots -> NEG_INF so exp()==0.
        Dummy rows get one j=0 slot with logit 0 so their softmax denominator
        stays finite (their h rows are zero, so the output row is 0)."""
        H = table.shape[1]
        tp = np.full((self.N + 1, H), NEG_INF, dtype=np.float16)
        tp[1:] = table
        arr = tp[self.gidx[core]]                    # [TOT, P, H]
        i_d, e_d = np.nonzero(self.rows_nodes[core] < 0)
        arr[self.off[i_d], e_d, :] = 0.0
        return np.ascontiguousarray(arr.transpose(1, 0, 2)).reshape(
            P, self.TOT * H)

    def stream_ald(self, table, core):
        """[128, wpc*H] f16: al_dst per (window, row). Dummy rows -> 0."""
        H = table.shape[1]
        tp = np.zeros((self.N + 1, H), dtype=np.float16)
        tp[1:] = table
        arr = tp[self.rows_nodes[core] + 1]          # [wpc, P, H]
        return np.ascontiguousarray(arr.transpose(1, 0, 2)).reshape(
            P, self.wpc * H)

    def stream_ald_exp(self, table, core):
        """[128, TOT*H] f16: al_dst replicated across each window's slots
        (slot grids are per-window blocks of D_i slots)."""
        H = table.shape[1]
        tp = np.zeros((self.N + 1, H), dtype=np.float16)
        tp[1:] = table
        arr = tp[self.rows_nodes[core] + 1]          # [wpc, P, H]
        rep = np.repeat(arr, self.D, axis=0)         # [TOT, P, H]
        return np.ascontiguousarray(rep.transpose(1, 0, 2)).reshape(
            P, self.TOT * H)

    def ident8(self):
        import ml_dtypes
        return np.eye(P, dtype=np.float32).astype(ml_dtypes.float8_e4m3)


# ------------------------------------------------------------------ builders

def _build_node(SH, c_in, m_h, m_al, elu, bias_in, bench_loop=1):
    """Per-node transform: hT = (elu?(xT+b)) @ w, alT = same @ wal.
    When m_h+m_al <= 128 the two matmuls merge into one.  The whole per-core
    panel is SBUF-resident: quarters stream in with fat DMAs, chunked matmuls
    write a staged output panel, and a few fat DMAs store it."""
    merged = (m_h + m_al) <= P
    M = m_h + m_al if merged else m_h
    QN = 4
    QS = SH // QN
    NQUAD = SH // (2 * CH)        # 2 al-chunks stack into one PSUM bank
    assert SH % QN == 0 and QS % CH == 0 and SH % (2 * CH) == 0
    nc = bass.Bass()
    xT = nc.dram_tensor("xT", [c_in, SH], F16, kind="ExternalInput")
    w = nc.dram_tensor("w", [c_in, M], F16, kind="ExternalInput")
    if not merged:
        assert m_al <= 32
        wal = nc.dram_tensor("wal", [c_in, 32], F16, kind="ExternalInput")
    if bias_in:
        bvec = nc.dram_tensor("bvec", [c_in, 1], F32, kind="ExternalInput")
    hT = nc.dram_tensor("hT", [M, SH], F16, kind="ExternalOutput")
    if not merged:
        # partition-stacked al panel: row 32k+r, col cq*CH+x holds
        # al[r] of chunk 2*cq+k (host unscrambles)
        alT = nc.dram_tensor("alT", [64, NQUAD * CH], F16,
                             kind="ExternalOutput")

    with tile.TileContext(nc) as tc:
        with (
            tc.tile_pool(name="const", bufs=1) as constp,
            tc.tile_pool(name="xin", bufs=2) as xinp,
            tc.tile_pool(name="hout", bufs=2) as houtp,
            tc.tile_pool(name="work", bufs=4) as workp,
            tc.tile_pool(name="psH", bufs=5, space="PSUM") as psH,
            tc.tile_pool(name="psA", bufs=3, space="PSUM") as psA,
        ):
            w_sb = constp.tile([c_in, M], F16)
            nc.sync.dma_start(out=w_sb[:], in_=w[:])
            if not merged:
                # wal host-padded to 32 cols (zeros) so every partition of
                # the stacked al PSUM region is written (no uninit reads)
                wal_sb = constp.tile([c_in, 32], F16)
                nc.sync.dma_start(out=wal_sb[:], in_=wal[:])
            if bias_in:
                b_sb = constp.tile([c_in, 1], F32)
                nc.sync.dma_start(out=b_sb[:], in_=bvec[:])

            def body(_iv=None):
                # every DMA rides SP: a queued DMA holds its issuing engine's
                # sequencer for the whole transfer, so ACT/DVE must stay clean
                xq = [xinp.tile([c_in, QS], F16, tag=f"x{q}", name=f"xq{q}")
                      for q in range(QN)]
                for q in range(QN):
                    nc.sync.dma_start(out=xq[q][:],
                                      in_=xT[:, q * QS:(q + 1) * QS])
                hq = [houtp.tile([M, QS], F16, tag=f"h{q}", name=f"hq{q}")
                      for q in range(QN)]
                if not merged:
                    alout = houtp.tile([64, NQUAD * CH], F16, tag="alo")
                quad = {}

                def qfront(q):
                    """Quarter-granular ELU stage A: one fat ACT exp."""
                    if not elu:
                        return None
                    rhs = xq[q][:]
                    if bias_in:
                        nc.vector.tensor_scalar(
                            rhs, rhs, b_sb[:, 0:1], None, OP.add)
                    et = workp.tile([c_in, QS], F16, tag="et")
                    nc.scalar.activation(et[:], rhs, AF.Exp)
                    return et

                def qback(q, et):
                    if elu:
                        # elu(x) = (min(exp(x),1) - 1) + max(x,0), all 2x DVE
                        mn = workp.tile([c_in, QS], F16, tag="mn")
                        nc.vector.tensor_scalar(
                            mn[:], et[:], 1.0, -1.0, OP.min, OP.add)
                        mx = workp.tile([c_in, QS], F16, tag="mx")
                        nc.vector.tensor_scalar(
                            mx[:], xq[q][:], 0.0, None, OP.max)
                        xe = workp.tile([c_in, QS], F16, tag="xe")
                        nc.vector.tensor_tensor(
                            out=xe[:], in0=mn[:], in1=mx[:], op=OP.add)
                        src = xe
                    else:
                        src = xq[q]
                    for j in range(QS // CH):
                        ci = q * (QS // CH) + j
                        qo = j * CH
                        rhs = src[:, qo:qo + CH]
                        ph = psH.tile([M, CH], F32, tag="ph")
                        nc.tensor.matmul(ph[:], w_sb[:], rhs,
                                         start=True, stop=True)
                        dve_copy = (ci % 3 == 2) if elu else (ci % 2 == 1)
                        if dve_copy:
                            nc.vector.tensor_copy(hq[q][:, qo:qo + CH],
                                                  ph[:])
                        else:
                            nc.scalar.activation(hq[q][:, qo:qo + CH],
                                                 ph[:], AF.Copy)
                        if not merged:
                            # stack 2 chunks' al outputs on partitions
                            # 0/32 of one PSUM bank -> 1 copy per pair
                            k = ci % 2
                            if k == 0:
                                quad["pa"] = psA.tile([64, CH], F32,
                                                      tag="paq", name="paq")
                            pa = quad["pa"]
                            nc.tensor.matmul(pa[32 * k:32 * k + 32, :],
                                             wal_sb[:], rhs,
                                             start=True, stop=True)
                            if k == 1:
                                cq = ci // 2
                                if cq % 2 == 0:
                                    nc.vector.tensor_copy(
                                        alout[:, cq * CH:(cq + 1) * CH],
                                        pa[:])
                                else:
                                    nc.scalar.activation(
                                        alout[:, cq * CH:(cq + 1) * CH],
                                        pa[:], AF.Copy)
                    nc.sync.dma_start(out=hT[:, q * QS:(q + 1) * QS],
                                      in_=hq[q][:])

                prev = None
                for q in range(QN):
                    et = qfront(q)
                    if prev is not None:
                        qback(*prev)
                    prev = (q, et)
                qback(*prev)
                if not merged:
                    nc.sync.dma_start(out=alT[:], in_=alout[:])

            if bench_loop > 1:
                with tc.For_i(0, bench_loop, 1) as _iv:
                    body(_iv)
            else:
                body()
    _finalize_kernel(nc)
    return nc


def _build_edge_g(D_list, groups, TOT, Cc, H, bias_out=False, elu_out=False,
                  ald_exp=False, bench_loop=1):
    """Edge aggregation over degree-sorted grids.  Per group of windows:
    one h[src] grid DMA, one DVE logit add per window, one ACT leaky-relu,
    one ACT exp into the message tile's trailing EB columns, one DVE
    multiply, then D accumulating identity matmuls per window.  Epilogues
    run one group late so no engine stalls on PSUM completion."""
    EB = 8
    SLOT = Cc + EB
    G = Cc // EB
    NW = len(D_list)
    GS = max(sd for _, _, _, sd in groups)
    NWmax = max(nw for _, nw, _, _ in groups)

    nc = bass.Bass()
    hsrc = nc.dram_tensor("hsrc", [P, TOT * Cc], F16, kind="ExternalInput")
    als = nc.dram_tensor("als", [P, TOT * H], F16, kind="ExternalInput")
    ald = nc.dram_tensor("ald", [P, (TOT if ald_exp else NW) * H], F16,
                         kind="ExternalInput")
    ident = nc.dram_tensor("ident", [P, P], F8, kind="ExternalInput")
    if bias_out:
        brep = nc.dram_tensor("brep", [P, Cc], F32, kind="ExternalInput")
    out = nc.dram_tensor("out", [NW * P, Cc], F16, kind="ExternalOutput")

    with tile.TileContext(nc) as tc:
        with (
            tc.tile_pool(name="const", bufs=1) as constp,
            tc.tile_pool(name="aldp", bufs=2) as aldp,
            tc.tile_pool(name="alg", bufs=3) as algp,
            tc.tile_pool(name="hs", bufs=3) as hsp,
            tc.tile_pool(name="za", bufs=2) as zap,
            tc.tile_pool(name="msg", bufs=3) as msgp,
            tc.tile_pool(name="epi", bufs=3) as epip,
            tc.tile_pool(name="og", bufs=2) as ogp,
            tc.tile_pool(name="psW", bufs=8, space="PSUM") as pswp,
        ):
            BSLOT = 512 // SLOT      # windows per PSUM bank
            ident_sb = constp.tile([P, P], F8)
            nc.scalar.dma_start(out=ident_sb[:], in_=ident[:])
            ebias_sb = constp.tile([P, 1], F32)
            nc.vector.memset(ebias_sb[:], EXP_BIAS)
            if bias_out:
                brep_sb = constp.tile([P, Cc], F32)
                nc.scalar.dma_start(out=brep_sb[:], in_=brep[:])

            pend = []

            def front(grp, ald_sb):
                """DMA + logit add + leaky-relu + exp for one group."""
                i0, nw, off0, sd = grp
                hs = hsp.tile([P, GS * Cc], F16, tag="hs")
                nc.sync.dma_start(out=hs[:, :sd * Cc],
                                  in_=hsrc[:, off0 * Cc:(off0 + sd) * Cc])
                alg = algp.tile([P, GS * H], F16, tag="alg")
                nc.sync.dma_start(out=alg[:, :sd * H],
                                  in_=als[:, off0 * H:(off0 + sd) * H])
                za = zap.tile([P, GS * H], F16, tag="za")
                if ald_exp:
                    # host replicated al_dst per slot: one add per group
                    adx = algp.tile([P, GS * H], F16, tag="adx")
                    nc.sync.dma_start(out=adx[:, :sd * H],
                                      in_=ald[:, off0 * H:(off0 + sd) * H])
                    nc.vector.tensor_tensor(out=za[:, :sd * H],
                                            in0=alg[:, :sd * H],
                                            in1=adx[:, :sd * H], op=OP.add)
                doff = 0
                for wl in range(nw) if not ald_exp else ():
                    D = int(D_list[i0 + wl])
                    o0 = doff * H
                    if H > 1:
                        av = alg[:, o0:o0 + D * H].rearrange(
                            "p (d h) -> p d h", d=D)
                        zv = za[:, o0:o0 + D * H].rearrange(
                            "p (d h) -> p d h", d=D)
                        ad = ald_sb[:, (i0 + wl) * H:(i0 + wl + 1) * H]
                        ab = bass.AP(ad.tensor, ad.offset,
                                     [ad.ap[0], [0, D], [1, H]])
                    else:
                        av = alg[:, o0:o0 + D]
                        zv = za[:, o0:o0 + D]
                        ad = ald_sb[:, i0 + wl:i0 + wl + 1]
                        ab = bass.AP(ad.tensor, ad.offset,
                                     [ad.ap[0], [0, D]])
                    nc.vector.tensor_tensor(out=zv, in0=av, in1=ab, op=OP.add)
                    doff += D
                nc.scalar.activation(za[:, :sd * H], za[:, :sd * H],
                                     AF.Prelu, alpha=NEG_SLOPE)
                msg = msgp.tile([P, GS * SLOT], F16, tag="msg")
                m3 = msg[:, :sd * SLOT].rearrange("p (d s) -> p d s", s=SLOT)
                eb_out = m3[:, :, Cc:Cc + EB]
                if H > 1:
                    e_in = za[:, :sd * H].rearrange("p (d h) -> p d h", d=sd)
                else:
                    z0 = za[:, :sd]
                    e_in = bass.AP(z0.tensor, z0.offset,
                                   [z0.ap[0], [1, sd], [0, EB]])
                nc.scalar.activation(eb_out, e_in, AF.Exp, bias=ebias_sb[:])
                return hs, msg

            def back(grp, st):
                """DVE message multiply + PE identity accumulation."""
                i0, nw, off0, sd = grp
                hs, msg = st
                m3 = msg[:, :sd * SLOT].rearrange("p (d s) -> p d s", s=SLOT)
                eb_out = m3[:, :, Cc:Cc + EB]
                mo = m3[:, :, 0:Cc].rearrange("p d (g h) -> p d g h", h=EB)
                hi = hs[:, :sd * Cc].rearrange(
                    "p (d g h) -> p d g h", d=sd, h=EB)
                ei = bass.AP(eb_out.tensor, eb_out.offset,
                             [eb_out.ap[0], eb_out.ap[1], [0, G], [1, EB]])
                nc.vector.tensor_tensor(out=mo, in0=hi, in1=ei, op=OP.mult)
                doff = 0
                bank = None
                for wl in range(nw):
                    D = int(D_list[i0 + wl])
                    if wl % BSLOT == 0:
                        bank = pswp.tile([P, 512], F32, tag="psw",
                                         name="pswbank")
                    sl = (wl % BSLOT) * SLOT
                    psw = bank[:, sl:sl + SLOT]
                    for j in range(D):
                        mv = msg[:, (doff + j) * SLOT:(doff + j + 1) * SLOT]
                        nc.tensor.matmul(psw, ident_sb[:], mv,
                                         start=(j == 0), stop=(j == D - 1))
                    pend.append(psw)
                    doff += D

            def epilogue(grp):
                """One f16 PSUM copy per window, then a single reciprocal +
                scale + output DMA for the whole group."""
                i0, nw, off0, sd = grp
                op_t = epip.tile([P, NWmax * SLOT], F16, tag="o1p")
                for wl in range(nw):
                    psw = pend.pop(0)
                    nc.scalar.activation(op_t[:, wl * SLOT:(wl + 1) * SLOT],
                                         psw, AF.Copy)
                opv = op_t[:, :nw * SLOT]
                rec = epip.tile([P, NWmax * EB], F16, tag="rec")
                rv = rec[:, :nw * EB].rearrange("p (w h) -> p w h", w=nw)
                dap = bass.AP(opv.tensor, opv.offset + Cc,
                              [opv.ap[0], [SLOT, nw], [1, EB]])
                with nc.allow_low_precision(
                        reason="softmax denominators are O(1)"):
                    nc.vector.reciprocal(rv, dap)
                og = ogp.tile([P, NWmax * Cc], F16, tag="og")
                o_in = bass.AP(opv.tensor, opv.offset,
                               [opv.ap[0], [SLOT, nw], [EB, G], [1, EB]])
                r0 = rec[:]
                r_b = bass.AP(r0.tensor, r0.offset,
                              [r0.ap[0], [EB, nw], [0, G], [1, EB]])
                oo = og[:, :nw * Cc].rearrange(
                    "p (w g h) -> p w g h", w=nw, h=EB)
                nc.vector.tensor_tensor(out=oo, in0=o_in, in1=r_b,
                                        op=OP.mult)
                if bias_out:     # layer bias: before the inter-layer elu
                    ov2 = og[:, :nw * Cc].rearrange("p (w c) -> p w c", w=nw)
                    b0 = brep_sb[:]
                    b_b = bass.AP(b0.tensor, b0.offset,
                                  [b0.ap[0], [0, nw], [1, Cc]])
                    nc.vector.tensor_tensor(out=ov2, in0=ov2, in1=b_b,
                                            op=OP.add)
                if elu_out:
                    # elu(x) = max(x,0) + (min(exp(x),1) - 1), in place on og
                    ogv = og[:, :nw * Cc]
                    et = epip.tile([P, NWmax * Cc], F16, tag="et")
                    etv = et[:, :nw * Cc]
                    nc.scalar.activation(etv, ogv, AF.Exp)
                    nc.vector.tensor_scalar(etv, etv, 1.0, -1.0,
                                            OP.min, OP.add)
                    nc.vector.scalar_tensor_tensor(ogv, ogv, 0.0, etv,
                                                   OP.max, OP.add)
                dr = out[i0 * P:(i0 + nw) * P, :].rearrange(
                    "(w e) c -> e w c", e=P)
                nc.scalar.dma_start(
                    out=dr,
                    in_=og[:, :nw * Cc].rearrange("p (w c) -> p w c", w=nw))

            def body(_iv=None):
                if not ald_exp:
                    ald_sb = aldp.tile([P, NW * H], F16, tag="ald")
                    nc.scalar.dma_start(out=ald_sb[:], in_=ald[:])
                else:
                    ald_sb = None
                pend.clear()
                sts = [None] * len(groups)
                for gi, grp in enumerate(groups):
                    sts[gi] = front(grp, ald_sb)
                    if gi >= 1:
                        back(groups[gi - 1], sts[gi - 1])
                        sts[gi - 1] = None
                    if gi >= 2:
                        epilogue(groups[gi - 2])
                ng = len(groups)
                back(groups[ng - 1], sts[ng - 1])
                if ng >= 2:
                    epilogue(groups[ng - 2])
                epilogue(groups[ng - 1])

            if bench_loop > 1:
                with tc.For_i(0, bench_loop, 1) as _iv:
                    body(_iv)
            else:
                body()
    _finalize_kernel(nc)
    return nc


# ------------------------------------------------------------------ runner

def _fold_att(W, a):
    heads, hid = a.shape
    return np.einsum("ihc,hc->ih", W.reshape(W.shape[0], heads, hid), a)


class _GatRunner:
    def __init__(self, n_cores=N_CORES):
        self.C = n_cores
        self._graph = None
        self._graph_key = None
        self._kernels = {}
        self.last_maps = {}

    def graph(self, edge_index, n_nodes):
        key = hash(np.asarray(edge_index).tobytes())
        if key != self._graph_key:
            self._graph = _Graph(edge_index, n_nodes, self.C)
            self._graph_key = key
            self._kernels.clear()
        return self._graph

    def kernel(self, name, bench_loop=1, **kw):
        key = (name, bench_loop, tuple(sorted(kw.items())))
        if key not in self._kernels:
            g = self._graph
            if name.startswith("P"):
                self._kernels[key] = _build_node(
                    g.shard_nodes, bench_loop=bench_loop, **kw)
            elif name == "E1":
                self._kernels[key] = _build_edge_g(
                    g.D, g.groups1, g.TOT, 128, 8,
                    bench_loop=bench_loop, **kw)
            else:
                self._kernels[key] = _build_edge_g(
                    g.D, g.groups2, g.TOT, 64, 1, ald_exp=True,
                    bench_loop=bench_loop, **kw)
        return self._kernels[key]

    def _run(self, name, nc, maps):
        self.last_maps[name] = maps
        res = run_bass_kernel_spmd(nc, maps, core_ids=list(range(self.C)))
        return res.results

    def run(self, x, edge_index, W1, a_src1, a_dst1, b1, W2, a_src2, a_dst2,
            b2):
        C = self.C
        N, IN_C = x.shape
        HEADS, HID = a_src1.shape
        HC = HEADS * HID
        OUT_C = W2.shape[1]
        g = self.graph(edge_index, N)
        SH = g.shard_nodes
        # (c,h)-interleaved channel order for the layer-1 hidden features:
        # col c*H+h of h1 holds math channel h*HID+c. Folded into W1's
        # columns (P0) and W2's rows (P2) on the host - pure permutation.
        perm = np.array([(j % HEADS) * HID + j // HEADS
                         for j in range(HC)], dtype=np.int64)

        # ---- P0: per-node h1 / logits --------------------------------
        xT_pad = np.zeros((IN_C, g.n_pad), dtype=np.float16)
        xT_pad[:, :N] = np.asarray(x, np.float32).T
        w1 = np.asarray(W1, np.float32)
        m_al = 2 * HEADS
        wal1 = np.zeros((IN_C, 32), dtype=np.float32)
        wal1[:, :m_al] = np.concatenate(
            [_fold_att(w1, np.asarray(a_src1, np.float32)),
             _fold_att(w1, np.asarray(a_dst1, np.float32))], axis=1)
        mapsP0 = [{"xT": np.ascontiguousarray(xT_pad[:, k * SH:(k + 1) * SH]),
                   "w": np.ascontiguousarray(w1[:, perm]).astype(np.float16),
                   "wal": wal1.astype(np.float16)} for k in range(C)]
        ncP0 = self.kernel("P0", c_in=IN_C, m_h=HC, m_al=m_al,
                           elu=False, bias_in=False)
        resP0 = self._run("P0", ncP0, mapsP0)
        h1 = np.ascontiguousarray(
            np.concatenate([r["hT"] for r in resP0], axis=1).T)[:N]
        # unscramble the partition-stacked al panel: row 32k+r, col cq*CH+x
        # holds al[r] of chunk 4*cq+k
        nq = SH // (2 * CH)
        al1 = np.concatenate(
            [r["alT"].reshape(2, 32, nq, CH)[:, :m_al]
             .transpose(1, 2, 0, 3).reshape(m_al, SH)
             for r in resP0], axis=1)                    # [16, Np]
        als1 = np.ascontiguousarray(al1[:HEADS, :N].T)
        ald1 = np.ascontiguousarray(al1[HEADS:, :N].T)

        # ---- E1: layer-1 edge aggregation + bias + ELU ---------------
        id8 = g.ident8()
        b1nz = bool(np.any(np.asarray(b1)))
        mapsE1 = []
        for k in range(C):
            m = {"hsrc": g.stream_h(h1, k),
                 "als": g.stream_als(als1, k),
                 "ald": g.stream_ald(ald1, k),
                 "ident": id8}
            if b1nz:
                m["brep"] = np.tile(
                    np.asarray(b1, np.float32)[perm], (P, 1))
            mapsE1.append(m)
        ncE1 = self.kernel("E1", bias_out=b1nz)
        resE1 = self._run("E1", ncE1, mapsE1)
        out1 = np.concatenate([r["out"] for r in resE1], axis=0)
        # rows of out1 are (core, slot, row) -> natural node rowmap
        rowmap = g.rows_nodes.reshape(-1)            # [C*wpc*P]

        # ---- P2: ELU + per-node h2 / logits --------------------------
        o1T = np.ascontiguousarray(out1.T)           # [HC, C*SH] f16
        w2 = np.asarray(W2, np.float32)
        wal2 = np.concatenate(
            [_fold_att(w2, np.asarray(a_src2, np.float32)),
             _fold_att(w2, np.asarray(a_dst2, np.float32))], axis=1)
        w2all = np.concatenate([w2[perm], wal2[perm]], axis=1)  # [HC, 66]
        mapsP2 = [
            {"xT": np.ascontiguousarray(o1T[:, k * SH:(k + 1) * SH]),
             "w": w2all.astype(np.float16)} for k in range(C)]
        # out1 already carries b1 (E1 bias_out); P2 applies the ELU
        ncP2 = self.kernel("P2", c_in=HC, m_h=OUT_C, m_al=2, elu=True,
                           bias_in=False)
        resP2 = self._run("P2", ncP2, mapsP2)
        h2al = np.concatenate([r["hT"] for r in resP2], axis=1)  # [66, Np]
        valid = rowmap >= 0
        vrows = rowmap[valid]
        h2 = np.zeros((N, OUT_C), dtype=np.float16)
        h2[vrows] = h2al[:OUT_C].T[valid]
        als2 = np.zeros((N, 1), dtype=np.float16)
        als2[vrows, 0] = h2al[OUT_C][valid]
        ald2 = np.zeros((N, 1), dtype=np.float16)
        ald2[vrows, 0] = h2al[OUT_C + 1][valid]

        # ---- E2: layer-2 edge aggregation ----------------------------
        b2nz = bool(np.any(np.asarray(b2)))
        mapsE2 = []
        for k in range(C):
            m = {"hsrc": g.stream_h(h2, k),
                 "als": g.stream_als(als2, k),
                 "ald": g.stream_ald_exp(ald2, k),
                 "ident": id8}
            if b2nz:
                m["brep"] = np.tile(np.asarray(b2, np.float32), (P, 1))
            mapsE2.append(m)
        ncE2 = self.kernel("E2", bias_out=b2nz)
        resE2 = self._run("E2", ncE2, mapsE2)
        out2 = np.concatenate([r["out"] for r in resE2], axis=0)
        out_full = np.zeros((N, OUT_C), dtype=np.float32)
        out_full[vrows] = out2[valid]
        return out_full


_RUNNER = _GatRunner()


def kernel(x, edge_index, W1, a_src1, a_dst1, b1, W2, a_src2, a_dst2, b2):
    """Full-input / full-output entry point. Returns [N, OUT_C] float32."""
    args = [np.asarray(v) for v in
            (x, edge_index, W1, a_src1, a_dst1, b1, W2, a_src2, a_dst2, b2)]
    return _RUNNER.run(*args).astype(np.float32)


# revision 43
# speedup vs baseline: 1.6581x; 1.0101x over previous
"""Trainium (trn2) Bass kernel for a 2-layer GAT over N=100k nodes / E=1.7M edges.

Strategy (degree-sorted edge grids + identity-stationary PE accumulation)
-------------------------------------------------------------------------
Nodes are sorted by in-degree on the host and packed into windows of 128
similar-degree destination nodes; windows are dealt round-robin across the 8
NeuronCores.  Each window's edges form a dense grid [128 nodes x D slots]
(D = max in-window degree, padded slots carry -inf logits so exp()==0), so
slot j of all 128 nodes is a 128-edge tile whose destination map is the
IDENTITY: the tensor engine accumulates the per-slot message tiles straight
into the window's PSUM bank with a never-changing fp8 identity stationary.
Degree sorting keeps grid padding at ~1.3%, and the one-hot selection stream
of the classic dst-sorted formulation (128 B/edge of pure index overhead)
disappears entirely.

Each GAT layer runs as TWO SPMD kernels with host-side index gathers (pure
permutations / casts - no host FLOPs) between them:

* node kernel (P0/P2): h = x @ W plus folded attention logits computed once
  per node (dense matmuls).  The full per-core input/output panels live in
  SBUF, loaded/stored with a handful of fat DMAs (per-chunk 1 KB/partition
  DMAs were latency-bound at ~140 GB/s); every DMA rides the SP queue since
  a queued DMA holds its issuing engine's sequencer for the whole transfer.
  P0's 16 logit rows stack two chunks per PSUM bank at partitions 0/32
  (tile_position) so one PSUM->SBUF copy drains two matmuls; P2 computes the
  inter-layer ELU as exp (one fat ACT op per quarter-panel, emitted a
  quarter ahead) + two 2x DVE ops, with PSUM copies balanced across ACT/DVE.
* edge kernel (E1/E2): streams h[src] grids (256/128 B per edge slot) and
  al_src logit grids (16/2 B); al_dst is a tiny per-window constant for E1
  and a host-replicated per-slot stream for E2 (one group-wide DVE add
  instead of 21 window-sized ones).  Windows are processed in groups
  (sum of D <= 96/192) software-pipelined three deep: group g's DMA +
  logits + leaky-relu + exp land while g-1 runs its DVE multiply + PE
  accumulation and g-2 runs its epilogue, so no engine ever stalls on
  another's latency.  ACT writes exp(z-4) into the message tile's trailing
  8 columns ((c,h)-interleaved broadcast for layer 1's 8 heads, an 8x
  replica for layer 2's single head so the DVE multiply keeps its
  packed-innermost 2x mode).  Epilogues drain each window's PSUM with a
  single f16 ACT copy, then one reciprocal + one scale per group.

Measured per-core DMA floor is ~343 GB/s on one queue / ~355 on two (HBM
fair share); the edge kernels stream ~62/~32 MB per core per inference and
run within ~15% of that floor.

Environment workarounds: this container's walrus build allows only ONE
semaphore wait per instruction (split onto nop carriers post-scheduling), and
the GPSIMD ucode libraries are absent (so no dma_gather/indirect-DMA fast
paths - hence the host-gather design).
"""
import numpy as np

import concourse.bass as bass
import concourse.mybir as mybir
import concourse.tile as tile
from concourse.bass_utils import run_bass_kernel_spmd

P = 128
F16 = mybir.dt.float16
F32 = mybir.dt.float32
F8 = mybir.dt.float8e4
AF = mybir.ActivationFunctionType
OP = mybir.AluOpType
NEG_SLOPE = 0.2
EXP_BIAS = -4.0     # exp(z + EXP_BIAS): constant shift cancels in softmax
NEG_INF = -60000.0  # pad-slot logit: exp(lrelu(.)+bias) underflows to 0
N_CORES = 8
EPS = 1e-30
CH = 448            # node-kernel matmul chunk (PSUM: 448*4B <= 2KB bank)
GCAP1, NWG1 = 96, 12     # E1 group capacity (sum of D's / max windows)
GCAP2, NWG2 = 192, 21    # E2 group capacity (smaller tiles -> fatter groups)

# ------------------------------------------------------------------ patches

_wsplit_counter = [0]


def _split_excess_waits(nc, max_waits=1):
    """This walrus build rejects >1 sem-wait per instruction ("Too many sync
    wait commands"). Move overflow waits onto same-engine nop carriers."""
    n_split = 0
    for f in nc.m.functions:
        for blk in f.blocks:
            changed = False
            out = []
            for inst in blk.instructions:
                si = inst.sync_info
                if si is not None and len(si.on_wait) > max_waits:
                    waits = list(si.on_wait)
                    keep = waits[len(waits) - max_waits:]
                    overflow = waits[: len(waits) - max_waits]
                    for i in range(0, len(overflow), max_waits):
                        _wsplit_counter[0] += 1
                        nop = mybir.InstNoOp(
                            name=f"I-wsplit-{_wsplit_counter[0]}", ins=[], outs=[])
                        nop.engine = inst.engine
                        nop.sync_info = mybir.SyncInfo(
                            on_wait=overflow[i: i + max_waits], on_update=[])
                        out.append(nop)
                    inst.sync_info = mybir.SyncInfo(
                        on_wait=keep, on_update=list(si.on_update))
                    changed = True
                    n_split += 1
                out.append(inst)
            if changed:
                blk.instructions = out
    return n_split


def _finalize_kernel(nc):
    import bass_rust as _bass_rust
    from concourse.library_config import all_libraries, standard
    from concourse.library_overlay import lower_extended_insts

    inst_type_to_lib_mask = {}
    for lib in all_libraries:
        for inst_type in lib.instructions:
            inst_type_to_lib_mask[inst_type] = inst_type_to_lib_mask.get(
                inst_type, 0) | (1 << lib.index)
    _bass_rust.insert_library_loads(
        nc, inst_type_to_lib_mask, len(all_libraries), standard.index)
    lower_extended_insts(nc)
    _split_excess_waits(nc)


# ------------------------------------------------------------------ host prep

class _Graph:
    """Degree-sorted grid preprocessing: sort nodes by in-degree, pack 128
    similar-degree nodes per window, deal windows round-robin across cores
    (slot i of every core shares one padded depth D_i so all cores run one
    identical SPMD program), and scatter each node's edges into its grid row.
    """

    def __init__(self, edge_index, n_nodes, n_cores):
        self.N = n_nodes
        self.C = n_cores
        src = np.asarray(edge_index[0], dtype=np.int64)
        dst = np.asarray(edge_index[1], dtype=np.int64)
        E = src.shape[0]

        deg = np.bincount(dst, minlength=n_nodes)
        order = np.argsort(deg, kind="stable")

        n_win_total = (n_nodes + P - 1) // P
        self.wpc = (n_win_total + n_cores - 1) // n_cores
        n_win = self.wpc * n_cores
        self.n_pad = n_win * P
        self.shard_nodes = self.wpc * P
        n_dummy = self.n_pad - n_nodes

        snode = np.full(self.n_pad, -1, dtype=np.int64)
        snode[n_dummy:] = order                      # ascending degree
        # rows_nodes[k][i, e] = natural node id at (core k, slot i, row e)
        self.rows_nodes = np.ascontiguousarray(
            snode.reshape(self.wpc, n_cores, P).transpose(1, 0, 2))

        wdeg = np.where(snode >= 0, deg[np.clip(snode, 0, None)], 0)
        wmax = wdeg.reshape(self.wpc, n_cores, P).max(axis=2)   # [wpc, cores]
        self.D = np.maximum(wmax.max(axis=1), 1).astype(np.int64)  # [wpc]
        self.off = np.concatenate([[0], np.cumsum(self.D)])
        self.TOT = int(self.D.sum())

        # position of each node in the sorted layout
        posq = np.empty(n_nodes, dtype=np.int64)
        posq[order] = np.arange(n_nodes) + n_dummy

        # scatter edges (dst-sorted, ranked within dst run) into grids
        perm = np.argsort(dst, kind="stable")
        src_s = src[perm]
        dst_s = dst[perm]
        bounds = np.searchsorted(dst_s, np.arange(n_nodes + 1))
        j_e = np.arange(E) - bounds[dst_s]           # rank within dst run
        q_e = posq[dst_s]
        g_e = q_e // P
        row_e = q_e % P
        core_e = g_e % n_cores
        slot_e = g_e // n_cores
        flat_e = self.off[slot_e] + j_e              # grid slot within [TOT]
        self.gidx = np.zeros((n_cores, self.TOT, P), dtype=np.int32)
        self.gidx[core_e, flat_e, row_e] = (src_s + 1).astype(np.int32)

        self.groups1 = self.make_groups(GCAP1, NWG1)
        self.groups2 = self.make_groups(GCAP2, NWG2)
        self.D_key = tuple(int(d) for d in self.D)

    def make_groups(self, gcap, nwg):
        """Window groups: sum(D) <= gcap, <= nwg windows per group."""
        groups = []
        i = 0
        while i < self.wpc:
            i0, sd, nw = i, 0, 0
            while (i < self.wpc and nw < nwg
                   and (nw == 0 or sd + int(self.D[i]) <= gcap)):
                sd += int(self.D[i])
                i += 1
                nw += 1
            groups.append((i0, nw, int(self.off[i0]), sd))
        return groups

    def stream_h(self, table, core):
        """[128, TOT*C] f16 grid gather: table rows by gidx (0 = zero pad)."""
        C = table.shape[1]
        tp = np.zeros((self.N + 1, C), dtype=np.float16)
        tp[1:] = table
        arr = tp[self.gidx[core]]                    # [TOT, P, C]
        return np.ascontiguousarray(arr.transpose(1, 0, 2)).reshape(
            P, self.TOT * C)

    def stream_als(self, table, core):
        """[128, TOT*H] f16: al_src grid; pad slots -> NEG_INF so exp()==0.
        Dummy rows get one j=0 slot with logit 0 so their softmax denominator
        stays finite (their h rows are zero, so the output row is 0)."""
        H = table.shape[1]
        tp = np.full((self.N + 1, H), NEG_INF, dtype=np.float16)
        tp[1:] = table
        arr = tp[self.gidx[core]]                    # [TOT, P, H]
        i_d, e_d = np.nonzero(self.rows_nodes[core] < 0)
        arr[self.off[i_d], e_d, :] = 0.0
        return np.ascontiguousarray(arr.transpose(1, 0, 2)).reshape(
            P, self.TOT * H)

    def stream_ald(self, table, core):
        """[128, wpc*H] f16: al_dst per (window, row). Dummy rows -> 0."""
        H = table.shape[1]
        tp = np.zeros((self.N + 1, H), dtype=np.float16)
        tp[1:] = table
        arr = tp[self.rows_nodes[core] + 1]          # [wpc, P, H]
        return np.ascontiguousarray(arr.transpose(1, 0, 2)).reshape(
            P, self.wpc * H)

    def stream_ald_exp(self, table, core):
        """[128, TOT*H] f16: al_dst replicated across each window's slots
        (slot grids are per-window blocks of D_i slots)."""
        H = table.shape[1]
        tp = np.zeros((self.N + 1, H), dtype=np.float16)
        tp[1:] = table
        arr = tp[self.rows_nodes[core] + 1]          # [wpc, P, H]
        rep = np.repeat(arr, self.D, axis=0)         # [TOT, P, H]
        return np.ascontiguousarray(rep.transpose(1, 0, 2)).reshape(
            P, self.TOT * H)

    def ident8(self):
        import ml_dtypes
        return np.eye(P, dtype=np.float32).astype(ml_dtypes.float8_e4m3)


# ------------------------------------------------------------------ builders

def _build_node(SH, c_in, m_h, m_al, elu, bias_in, bench_loop=1):
    """Per-node transform: hT = (elu?(xT+b)) @ w, alT = same @ wal.
    When m_h+m_al <= 128 the two matmuls merge into one.  The whole per-core
    panel is SBUF-resident: quarters stream in with fat DMAs, chunked matmuls
    write a staged output panel, and a few fat DMAs store it."""
    merged = (m_h + m_al) <= P
    M = m_h + m_al if merged else m_h
    QN = 4
    QS = SH // QN
    NQUAD = SH // (2 * CH)        # 2 al-chunks stack into one PSUM bank
    assert SH % QN == 0 and QS % CH == 0 and SH % (2 * CH) == 0
    nc = bass.Bass()
    xT = nc.dram_tensor("xT", [c_in, SH], F16, kind="ExternalInput")
    w = nc.dram_tensor("w", [c_in, M], F16, kind="ExternalInput")
    if not merged:
        assert m_al <= 32
        wal = nc.dram_tensor("wal", [c_in, 32], F16, kind="ExternalInput")
    if bias_in:
        bvec = nc.dram_tensor("bvec", [c_in, 1], F32, kind="ExternalInput")
    hT = nc.dram_tensor("hT", [M, SH], F16, kind="ExternalOutput")
    if not merged:
        # partition-stacked al panel: row 32k+r, col cq*CH+x holds
        # al[r] of chunk 2*cq+k (host unscrambles)
        alT = nc.dram_tensor("alT", [64, NQUAD * CH], F16,
                             kind="ExternalOutput")

    with tile.TileContext(nc) as tc:
        with (
            tc.tile_pool(name="const", bufs=1) as constp,
            tc.tile_pool(name="xin", bufs=2) as xinp,
            tc.tile_pool(name="hout", bufs=2) as houtp,
            tc.tile_pool(name="work", bufs=4) as workp,
            tc.tile_pool(name="psH", bufs=5, space="PSUM") as psH,
            tc.tile_pool(name="psA", bufs=3, space="PSUM") as psA,
        ):
            w_sb = constp.tile([c_in, M], F16)
            nc.sync.dma_start(out=w_sb[:], in_=w[:])
            if not merged:
                # wal host-padded to 32 cols (zeros) so every partition of
                # the stacked al PSUM region is written (no uninit reads)
                wal_sb = constp.tile([c_in, 32], F16)
                nc.sync.dma_start(out=wal_sb[:], in_=wal[:])
            if bias_in:
                b_sb = constp.tile([c_in, 1], F32)
                nc.sync.dma_start(out=b_sb[:], in_=bvec[:])

            def body(_iv=None):
                # every DMA rides SP: a queued DMA holds its issuing engine's
                # sequencer for the whole transfer, so ACT/DVE must stay clean
                xq = [xinp.tile([c_in, QS], F16, tag=f"x{q}", name=f"xq{q}")
                      for q in range(QN)]
                for q in range(QN):
                    nc.sync.dma_start(out=xq[q][:],
                                      in_=xT[:, q * QS:(q + 1) * QS])
                hq = [houtp.tile([M, QS], F16, tag=f"h{q}", name=f"hq{q}")
                      for q in range(QN)]
                if not merged:
                    alout = houtp.tile([64, NQUAD * CH], F16, tag="alo")
                quad = {}

                def qfront(q):
                    """Quarter-granular ELU stage A: one fat ACT exp."""
                    if not elu:
                        return None
                    rhs = xq[q][:]
                    if bias_in:
                        nc.vector.tensor_scalar(
                            rhs, rhs, b_sb[:, 0:1], None, OP.add)
                    et = workp.tile([c_in, QS], F16, tag="et")
                    nc.scalar.activation(et[:], rhs, AF.Exp)
                    return et

                def qback(q, et):
                    if elu:
                        # elu(x) = (min(exp(x),1) - 1) + max(x,0), all 2x DVE
                        mn = workp.tile([c_in, QS], F16, tag="mn")
                        nc.vector.tensor_scalar(
                            mn[:], et[:], 1.0, -1.0, OP.min, OP.add)
                        mx = workp.tile([c_in, QS], F16, tag="mx")
                        nc.vector.tensor_scalar(
                            mx[:], xq[q][:], 0.0, None, OP.max)
                        xe = workp.tile([c_in, QS], F16, tag="xe")
                        nc.vector.tensor_tensor(
                            out=xe[:], in0=mn[:], in1=mx[:], op=OP.add)
                        src = xe
                    else:
                        src = xq[q]
                    for j in range(QS // CH):
                        ci = q * (QS // CH) + j
                        qo = j * CH
                        rhs = src[:, qo:qo + CH]
                        ph = psH.tile([M, CH], F32, tag="ph")
                        nc.tensor.matmul(ph[:], w_sb[:], rhs,
                                         start=True, stop=True)
                        dve_copy = (ci % 7 < 3) if elu else (ci % 2 == 1)
                        if dve_copy:
                            nc.vector.tensor_copy(hq[q][:, qo:qo + CH],
                                                  ph[:])
                        else:
                            nc.scalar.activation(hq[q][:, qo:qo + CH],
                                                 ph[:], AF.Copy)
                        if not merged:
                            # stack 2 chunks' al outputs on partitions
                            # 0/32 of one PSUM bank -> 1 copy per pair
                            k = ci % 2
                            if k == 0:
                                quad["pa"] = psA.tile([64, CH], F32,
                                                      tag="paq", name="paq")
                            pa = quad["pa"]
                            nc.tensor.matmul(pa[32 * k:32 * k + 32, :],
                                             wal_sb[:], rhs,
                                             start=True, stop=True)
                            if k == 1:
                                cq = ci // 2
                                if cq % 2 == 0:
                                    nc.vector.tensor_copy(
                                        alout[:, cq * CH:(cq + 1) * CH],
                                        pa[:])
                                else:
                                    nc.scalar.activation(
                                        alout[:, cq * CH:(cq + 1) * CH],
                                        pa[:], AF.Copy)
                    nc.sync.dma_start(out=hT[:, q * QS:(q + 1) * QS],
                                      in_=hq[q][:])

                prev = None
                for q in range(QN):
                    et = qfront(q)
                    if prev is not None:
                        qback(*prev)
                    prev = (q, et)
                qback(*prev)
                if not merged:
                    nc.sync.dma_start(out=alT[:], in_=alout[:])

            if bench_loop > 1:
                with tc.For_i(0, bench_loop, 1) as _iv:
                    body(_iv)
            else:
                body()
    _finalize_kernel(nc)
    return nc


def _build_edge_g(D_list, groups, TOT, Cc, H, bias_out=False, elu_out=False,
                  ald_exp=False, bench_loop=1):
    """Edge aggregation over degree-sorted grids.  Per group of windows:
    one h[src] grid DMA, one DVE logit add per window, one ACT leaky-relu,
    one ACT exp into the message tile's trailing EB columns, one DVE
    multiply, then D accumulating identity matmuls per window.  Epilogues
    run one group late so no engine stalls on PSUM completion."""
    EB = 8
    SLOT = Cc + EB
    G = Cc // EB
    NW = len(D_list)
    GS = max(sd for _, _, _, sd in groups)
    NWmax = max(nw for _, nw, _, _ in groups)

    nc = bass.Bass()
    hsrc = nc.dram_tensor("hsrc", [P, TOT * Cc], F16, kind="ExternalInput")
    als = nc.dram_tensor("als", [P, TOT * H], F16, kind="ExternalInput")
    ald = nc.dram_tensor("ald", [P, (TOT if ald_exp else NW) * H], F16,
                         kind="ExternalInput")
    ident = nc.dram_tensor("ident", [P, P], F8, kind="ExternalInput")
    if bias_out:
        brep = nc.dram_tensor("brep", [P, Cc], F32, kind="ExternalInput")
    out = nc.dram_tensor("out", [NW * P, Cc], F16, kind="ExternalOutput")

    with tile.TileContext(nc) as tc:
        with (
            tc.tile_pool(name="const", bufs=1) as constp,
            tc.tile_pool(name="aldp", bufs=2) as aldp,
            tc.tile_pool(name="alg", bufs=3) as algp,
            tc.tile_pool(name="hs", bufs=3) as hsp,
            tc.tile_pool(name="za", bufs=3) as zap,
            tc.tile_pool(name="msg", bufs=3) as msgp,
            tc.tile_pool(name="epi", bufs=3) as epip,
            tc.tile_pool(name="og", bufs=2) as ogp,
            tc.tile_pool(name="psW", bufs=8, space="PSUM") as pswp,
        ):
            BSLOT = 512 // SLOT      # windows per PSUM bank
            ident_sb = constp.tile([P, P], F8)
            nc.scalar.dma_start(out=ident_sb[:], in_=ident[:])
            ebias_sb = constp.tile([P, 1], F32)
            nc.vector.memset(ebias_sb[:], EXP_BIAS)
            if bias_out:
                brep_sb = constp.tile([P, Cc], F32)
                nc.scalar.dma_start(out=brep_sb[:], in_=brep[:])

            pend = []

            def front(grp, ald_sb):
                """DMA + logit add + leaky-relu + exp for one group."""
                i0, nw, off0, sd = grp
                hs = hsp.tile([P, GS * Cc], F16, tag="hs")
                nc.sync.dma_start(out=hs[:, :sd * Cc],
                                  in_=hsrc[:, off0 * Cc:(off0 + sd) * Cc])
                alg = algp.tile([P, GS * H], F16, tag="alg")
                nc.sync.dma_start(out=alg[:, :sd * H],
                                  in_=als[:, off0 * H:(off0 + sd) * H])
                za = zap.tile([P, GS * H], F16, tag="za")
                if ald_exp:
                    # host replicated al_dst per slot: one add per group
                    adx = algp.tile([P, GS * H], F16, tag="adx")
                    nc.sync.dma_start(out=adx[:, :sd * H],
                                      in_=ald[:, off0 * H:(off0 + sd) * H])
                    nc.vector.tensor_tensor(out=za[:, :sd * H],
                                            in0=alg[:, :sd * H],
                                            in1=adx[:, :sd * H], op=OP.add)
                doff = 0
                for wl in range(nw) if not ald_exp else ():
                    D = int(D_list[i0 + wl])
                    o0 = doff * H
                    if H > 1:
                        av = alg[:, o0:o0 + D * H].rearrange(
                            "p (d h) -> p d h", d=D)
                        zv = za[:, o0:o0 + D * H].rearrange(
                            "p (d h) -> p d h", d=D)
                        ad = ald_sb[:, (i0 + wl) * H:(i0 + wl + 1) * H]
                        ab = bass.AP(ad.tensor, ad.offset,
                                     [ad.ap[0], [0, D], [1, H]])
                    else:
                        av = alg[:, o0:o0 + D]
                        zv = za[:, o0:o0 + D]
                        ad = ald_sb[:, i0 + wl:i0 + wl + 1]
                        ab = bass.AP(ad.tensor, ad.offset,
                                     [ad.ap[0], [0, D]])
                    nc.vector.tensor_tensor(out=zv, in0=av, in1=ab, op=OP.add)
                    doff += D
                nc.scalar.activation(za[:, :sd * H], za[:, :sd * H],
                                     AF.Prelu, alpha=NEG_SLOPE)
                msg = msgp.tile([P, GS * SLOT], F16, tag="msg")
                m3 = msg[:, :sd * SLOT].rearrange("p (d s) -> p d s", s=SLOT)
                eb_out = m3[:, :, Cc:Cc + EB]
                if H > 1:
                    e_in = za[:, :sd * H].rearrange("p (d h) -> p d h", d=sd)
                else:
                    z0 = za[:, :sd]
                    e_in = bass.AP(z0.tensor, z0.offset,
                                   [z0.ap[0], [1, sd], [0, EB]])
                nc.scalar.activation(eb_out, e_in, AF.Exp, bias=ebias_sb[:])
                return hs, msg

            def back(grp, st):
                """DVE message multiply + PE identity accumulation."""
                i0, nw, off0, sd = grp
                hs, msg = st
                m3 = msg[:, :sd * SLOT].rearrange("p (d s) -> p d s", s=SLOT)
                eb_out = m3[:, :, Cc:Cc + EB]
                mo = m3[:, :, 0:Cc].rearrange("p d (g h) -> p d g h", h=EB)
                hi = hs[:, :sd * Cc].rearrange(
                    "p (d g h) -> p d g h", d=sd, h=EB)
                ei = bass.AP(eb_out.tensor, eb_out.offset,
                             [eb_out.ap[0], eb_out.ap[1], [0, G], [1, EB]])
                nc.vector.tensor_tensor(out=mo, in0=hi, in1=ei, op=OP.mult)
                doff = 0
                bank = None
                for wl in range(nw):
                    D = int(D_list[i0 + wl])
                    if wl % BSLOT == 0:
                        bank = pswp.tile([P, 512], F32, tag="psw",
                                         name="pswbank")
                    sl = (wl % BSLOT) * SLOT
                    psw = bank[:, sl:sl + SLOT]
                    for j in range(D):
                        mv = msg[:, (doff + j) * SLOT:(doff + j + 1) * SLOT]
                        nc.tensor.matmul(psw, ident_sb[:], mv,
                                         start=(j == 0), stop=(j == D - 1))
                    pend.append(psw)
                    doff += D

            def epilogue(grp):
                """One f16 PSUM copy per window, then a single reciprocal +
                scale + output DMA for the whole group."""
                i0, nw, off0, sd = grp
                op_t = epip.tile([P, NWmax * SLOT], F16, tag="o1p")
                for wl in range(nw):
                    psw = pend.pop(0)
                    nc.scalar.activation(op_t[:, wl * SLOT:(wl + 1) * SLOT],
                                         psw, AF.Copy)
                opv = op_t[:, :nw * SLOT]
                rec = epip.tile([P, NWmax * EB], F16, tag="rec")
                rv = rec[:, :nw * EB].rearrange("p (w h) -> p w h", w=nw)
                dap = bass.AP(opv.tensor, opv.offset + Cc,
                              [opv.ap[0], [SLOT, nw], [1, EB]])
                with nc.allow_low_precision(
                        reason="softmax denominators are O(1)"):
                    nc.vector.reciprocal(rv, dap)
                og = ogp.tile([P, NWmax * Cc], F16, tag="og")
                o_in = bass.AP(opv.tensor, opv.offset,
                               [opv.ap[0], [SLOT, nw], [EB, G], [1, EB]])
                r0 = rec[:]
                r_b = bass.AP(r0.tensor, r0.offset,
                              [r0.ap[0], [EB, nw], [0, G], [1, EB]])
                oo = og[:, :nw * Cc].rearrange(
                    "p (w g h) -> p w g h", w=nw, h=EB)
                nc.vector.tensor_tensor(out=oo, in0=o_in, in1=r_b,
                                        op=OP.mult)
                if bias_out:     # layer bias: before the inter-layer elu
                    ov2 = og[:, :nw * Cc].rearrange("p (w c) -> p w c", w=nw)
                    b0 = brep_sb[:]
                    b_b = bass.AP(b0.tensor, b0.offset,
                                  [b0.ap[0], [0, nw], [1, Cc]])
                    nc.vector.tensor_tensor(out=ov2, in0=ov2, in1=b_b,
                                            op=OP.add)
                if elu_out:
                    # elu(x) = max(x,0) + (min(exp(x),1) - 1), in place on og
                    ogv = og[:, :nw * Cc]
                    et = epip.tile([P, NWmax * Cc], F16, tag="et")
                    etv = et[:, :nw * Cc]
                    nc.scalar.activation(etv, ogv, AF.Exp)
                    nc.vector.tensor_scalar(etv, etv, 1.0, -1.0,
                                            OP.min, OP.add)
                    nc.vector.scalar_tensor_tensor(ogv, ogv, 0.0, etv,
                                                   OP.max, OP.add)
                dr = out[i0 * P:(i0 + nw) * P, :].rearrange(
                    "(w e) c -> e w c", e=P)
                nc.scalar.dma_start(
                    out=dr,
                    in_=og[:, :nw * Cc].rearrange("p (w c) -> p w c", w=nw))

            def body(_iv=None):
                if not ald_exp:
                    ald_sb = aldp.tile([P, NW * H], F16, tag="ald")
                    nc.scalar.dma_start(out=ald_sb[:], in_=ald[:])
                else:
                    ald_sb = None
                pend.clear()
                sts = [None] * len(groups)
                for gi, grp in enumerate(groups):
                    sts[gi] = front(grp, ald_sb)
                    if gi >= 1:
                        back(groups[gi - 1], sts[gi - 1])
                        sts[gi - 1] = None
                    if gi >= 2:
                        epilogue(groups[gi - 2])
                ng = len(groups)
                back(groups[ng - 1], sts[ng - 1])
                if ng >= 2:
                    epilogue(groups[ng - 2])
                epilogue(groups[ng - 1])

            if bench_loop > 1:
                with tc.For_i(0, bench_loop, 1) as _iv:
                    body(_iv)
            else:
                body()
    _finalize_kernel(nc)
    return nc


# ------------------------------------------------------------------ runner

def _fold_att(W, a):
    heads, hid = a.shape
    return np.einsum("ihc,hc->ih", W.reshape(W.shape[0], heads, hid), a)


class _GatRunner:
    def __init__(self, n_cores=N_CORES):
        self.C = n_cores
        self._graph = None
        self._graph_key = None
        self._kernels = {}
        self.last_maps = {}

    def graph(self, edge_index, n_nodes):
        key = hash(np.asarray(edge_index).tobytes())
        if key != self._graph_key:
            self._graph = _Graph(edge_index, n_nodes, self.C)
            self._graph_key = key
            self._kernels.clear()
        return self._graph

    def kernel(self, name, bench_loop=1, **kw):
        key = (name, bench_loop, tuple(sorted(kw.items())))
        if key not in self._kernels:
            g = self._graph
            if name.startswith("P"):
                self._kernels[key] = _build_node(
                    g.shard_nodes, bench_loop=bench_loop, **kw)
            elif name == "E1":
                self._kernels[key] = _build_edge_g(
                    g.D, g.groups1, g.TOT, 128, 8,
                    bench_loop=bench_loop, **kw)
            else:
                self._kernels[key] = _build_edge_g(
                    g.D, g.groups2, g.TOT, 64, 1, ald_exp=True,
                    bench_loop=bench_loop, **kw)
        return self._kernels[key]

    def _run(self, name, nc, maps):
        self.last_maps[name] = maps
        res = run_bass_kernel_spmd(nc, maps, core_ids=list(range(self.C)))
        return res.results

    def run(self, x, edge_index, W1, a_src1, a_dst1, b1, W2, a_src2, a_dst2,
            b2):
        C = self.C
        N, IN_C = x.shape
        HEADS, HID = a_src1.shape
        HC = HEADS * HID
        OUT_C = W2.shape[1]
        g = self.graph(edge_index, N)
        SH = g.shard_nodes
        # (c,h)-interleaved channel order for the layer-1 hidden features:
        # col c*H+h of h1 holds math channel h*HID+c. Folded into W1's
        # columns (P0) and W2's rows (P2) on the host - pure permutation.
        perm = np.array([(j % HEADS) * HID + j // HEADS
                         for j in range(HC)], dtype=np.int64)

        # ---- P0: per-node h1 / logits --------------------------------
        xT_pad = np.zeros((IN_C, g.n_pad), dtype=np.float16)
        xT_pad[:, :N] = np.asarray(x, np.float32).T
        w1 = np.asarray(W1, np.float32)
        m_al = 2 * HEADS
        wal1 = np.zeros((IN_C, 32), dtype=np.float32)
        wal1[:, :m_al] = np.concatenate(
            [_fold_att(w1, np.asarray(a_src1, np.float32)),
             _fold_att(w1, np.asarray(a_dst1, np.float32))], axis=1)
        mapsP0 = [{"xT": np.ascontiguousarray(xT_pad[:, k * SH:(k + 1) * SH]),
                   "w": np.ascontiguousarray(w1[:, perm]).astype(np.float16),
                   "wal": wal1.astype(np.float16)} for k in range(C)]
        ncP0 = self.kernel("P0", c_in=IN_C, m_h=HC, m_al=m_al,
                           elu=False, bias_in=False)
        resP0 = self._run("P0", ncP0, mapsP0)
        h1 = np.ascontiguousarray(
            np.concatenate([r["hT"] for r in resP0], axis=1).T)[:N]
        # unscramble the partition-stacked al panel: row 32k+r, col cq*CH+x
        # holds al[r] of chunk 4*cq+k
        nq = SH // (2 * CH)
        al1 = np.concatenate(
            [r["alT"].reshape(2, 32, nq, CH)[:, :m_al]
             .transpose(1, 2, 0, 3).reshape(m_al, SH)
             for r in resP0], axis=1)                    # [16, Np]
        als1 = np.ascontiguousarray(al1[:HEADS, :N].T)
        ald1 = np.ascontiguousarray(al1[HEADS:, :N].T)

        # ---- E1: layer-1 edge aggregation + bias + ELU ---------------
        id8 = g.ident8()
        b1nz = bool(np.any(np.asarray(b1)))
        mapsE1 = []
        for k in range(C):
            m = {"hsrc": g.stream_h(h1, k),
                 "als": g.stream_als(als1, k),
                 "ald": g.stream_ald(ald1, k),
                 "ident": id8}
            if b1nz:
                m["brep"] = np.tile(
                    np.asarray(b1, np.float32)[perm], (P, 1))
            mapsE1.append(m)
        ncE1 = self.kernel("E1", bias_out=b1nz)
        resE1 = self._run("E1", ncE1, mapsE1)
        out1 = np.concatenate([r["out"] for r in resE1], axis=0)
        # rows of out1 are (core, slot, row) -> natural node rowmap
        rowmap = g.rows_nodes.reshape(-1)            # [C*wpc*P]

        # ---- P2: ELU + per-node h2 / logits --------------------------
        o1T = np.ascontiguousarray(out1.T)           # [HC, C*SH] f16
        w2 = np.asarray(W2, np.float32)
        wal2 = np.concatenate(
            [_fold_att(w2, np.asarray(a_src2, np.float32)),
             _fold_att(w2, np.asarray(a_dst2, np.float32))], axis=1)
        w2all = np.concatenate([w2[perm], wal2[perm]], axis=1)  # [HC, 66]
        mapsP2 = [
            {"xT": np.ascontiguousarray(o1T[:, k * SH:(k + 1) * SH]),
             "w": w2all.astype(np.float16)} for k in range(C)]
        # out1 already carries b1 (E1 bias_out); P2 applies the ELU
        ncP2 = self.kernel("P2", c_in=HC, m_h=OUT_C, m_al=2, elu=True,
                           bias_in=False)
        resP2 = self._run("P2", ncP2, mapsP2)
        h2al = np.concatenate([r["hT"] for r in resP2], axis=1)  # [66, Np]
        valid = rowmap >= 0
        vrows = rowmap[valid]
        h2 = np.zeros((N, OUT_C), dtype=np.float16)
        h2[vrows] = h2al[:OUT_C].T[valid]
        als2 = np.zeros((N, 1), dtype=np.float16)
        als2[vrows, 0] = h2al[OUT_C][valid]
        ald2 = np.zeros((N, 1), dtype=np.float16)
        ald2[vrows, 0] = h2al[OUT_C + 1][valid]

        # ---- E2: layer-2 edge aggregation ----------------------------
        b2nz = bool(np.any(np.asarray(b2)))
        mapsE2 = []
        for k in range(C):
            m = {"hsrc": g.stream_h(h2, k),
                 "als": g.stream_als(als2, k),
                 "ald": g.stream_ald_exp(ald2, k),
                 "ident": id8}
            if b2nz:
                m["brep"] = np.tile(np.asarray(b2, np.float32), (P, 1))
            mapsE2.append(m)
        ncE2 = self.kernel("E2", bias_out=b2nz)
        resE2 = self._run("E2", ncE2, mapsE2)
        out2 = np.concatenate([r["out"] for r in resE2], axis=0)
        out_full = np.zeros((N, OUT_C), dtype=np.float32)
        out_full[vrows] = out2[valid]
        return out_full


_RUNNER = _GatRunner()


def kernel(x, edge_index, W1, a_src1, a_dst1, b1, W2, a_src2, a_dst2, b2):
    """Full-input / full-output entry point. Returns [N, OUT_C] float32."""
    args = [np.asarray(v) for v in
            (x, edge_index, W1, a_src1, a_dst1, b1, W2, a_src2, a_dst2, b2)]
    return _RUNNER.run(*args).astype(np.float32)
